# revision 3
# baseline (speedup 1.0000x reference)
"""CrissCrossAttention Trainium2 kernel.

Sharding: 8 cores = 4 samples x 2 row-halves. Each core gets its sample with
rows rolled so its 49 rows sit at local rows [0, 49) (column attention is
permutation-invariant over the column index, so rolling is safe and keeps the
SPMD program identical across cores).

Math (per core, local rows j in [0,49), cols x in [0,97)):
  q|k = Wqk @ x + bqk                       (fp16 matmuls, PSUM fp32)
  E_row[(j,x), i] = q(j,x).k(j,i)           (row energies, both orientations)
  E_col[(j,x), i] = q(j,x).k(i,x)           (col energies, both orientations)
  P = exp(E) unnormalized (no max-shift; |E| <~ 45 fits fp32), col diag i==j
  zeroed via affine_select. S = sum P (row incl self + col excl self).
  U = P_row @ X_row + P_col @ X_col         (bf16 aggregation of raw x)
  out = (Wv @ U) * (gamma/S) + (x + gamma*bv)   (Wv after aggregation;
  softmax scale and bias folded in at the end - exact by linearity)
"""

import os
import numpy as np
import ml_dtypes

import concourse.bacc as bacc
import concourse.bass as bass
import concourse.tile as tile
import concourse.mybir as mybir
from concourse.bass_utils import run_bass_kernel_spmd
from concourse.masks import make_identity

F32 = mybir.dt.float32
F16 = mybir.dt.float16
BF16 = mybir.dt.bfloat16
AF = mybir.ActivationFunctionType
AX = mybir.AxisListType
ALU = mybir.AluOpType

C = 512
CC = 4          # 4 chunks of 128 channels
CQ = 64
H = W = 97
NPIX = H * W    # 9409
R = 49          # rows per core (halves overlap at row 48)
PX = R * W      # 4753
GP = 4          # process group (cols / rows per softmax group)
GD = 8          # dma group for xt slices

_cache = {}
last_results = None


def _groups(total, g):
    out = []
    i = 0
    while i < total:
        out.append((i, min(g, total - i)))
        i += g
    return out


def _build(gamma: float):
    nc = bacc.Bacc("TRN2", target_bir_lowering=False, debug=False,
                   enable_asserts=False)

    xc = nc.dram_tensor("xc", [128, CC, NPIX], F16, kind="ExternalInput")
    xt = nc.dram_tensor("xt", [NPIX, C], BF16, kind="ExternalInput")
    xr = nc.dram_tensor("xr", [R, W, C], BF16, kind="ExternalInput")
    wqk = nc.dram_tensor("wqk", [128, CC, 128], F16, kind="ExternalInput")
    bqk = nc.dram_tensor("bqk", [1, 128], F16, kind="ExternalInput")
    onesd = nc.dram_tensor("onesd", [1, 512], F16, kind="ExternalInput")
    wv = nc.dram_tensor("wv", [128, CC, C], BF16, kind="ExternalInput")
    out = nc.dram_tensor("out", [R, W, C], F32, kind="ExternalOutput")

    xt_col = xt.ap().rearrange("(x y) c -> y x c", y=W)   # [y, x, c] part=y
    xt_row = xt.ap().rearrange("(x y) c -> x y c", y=W)   # [i=x, j=y, c] part=i
    xr_v = xr.ap().rearrange("j x c -> x j c")            # part = x
    out_v = out.ap().rearrange("j x c -> x j c")

    with tile.TileContext(nc) as tc:
        with (
            tc.tile_pool(name="singles", bufs=1) as singles,
            tc.tile_pool(name="xcp", bufs=3) as xcp,
            tc.tile_pool(name="xtcp", bufs=3) as xtcp,
            tc.tile_pool(name="xtrp", bufs=3) as xtrp,
            tc.tile_pool(name="xrp", bufs=3) as xrp,
            tc.tile_pool(name="outp", bufs=2) as outp,
            tc.tile_pool(name="scrp", bufs=2) as scrp,
            tc.tile_pool(name="ptp", bufs=4) as ptp,
            tc.tile_pool(name="cgp", bufs=3) as cgp,
            tc.tile_pool(name="smallp", bufs=4) as smallp,
            tc.tile_pool(name="ps_eA", bufs=2, space="PSUM") as ps_eA,
            tc.tile_pool(name="ps_eT", bufs=2, space="PSUM") as ps_eT,
            tc.tile_pool(name="ps_agg", bufs=2, space="PSUM") as ps_agg,
            tc.tile_pool(name="ps_big", bufs=2, space="PSUM") as ps_big,
        ):
            # ---- constants ----
            wqk_sb = singles.tile([128, CC, 128], F16)
            nc.sync.dma_start(out=wqk_sb, in_=wqk.ap())
            bqk_sb = singles.tile([1, 128], F16)
            nc.sync.dma_start(out=bqk_sb, in_=bqk.ap())
            ones_sb = singles.tile([1, 512], F16)
            nc.sync.dma_start(out=ones_sb, in_=onesd.ap())
            wv_sb = singles.tile([128, CC, C], BF16)
            nc.sync.dma_start(out=wv_sb, in_=wv.ap())
            ident = singles.tile([64, 64], F32)
            make_identity(nc, ident)

            qk_sb = singles.tile([128, NPIX], F16)
            k_sb = singles.tile([CQ, NPIX], F16)
            z_sb = singles.tile([128, CC, PX], BF16)
            srow = singles.tile([W, R], F32)
            scol = singles.tile([R, W], F32)
            scolT = singles.tile([W, R], F32)

            qk3 = qk_sb.rearrange("p (y x) -> p y x", x=W)
            k3 = k_sb.rearrange("p (y x) -> p y x", x=W)
            z4 = z_sb.rearrange("p c (y x) -> p c y x", x=W)

            # ---- projections: qk = [wq|wk] @ x + bqk (fp16) ----
            for p0, n in _groups(NPIX, 512):
                xc_t = xcp.tile([128, CC, 512], F16)
                nc.sync.dma_start(out=xc_t[:, :, :n], in_=xc.ap()[:, :, p0:p0 + n])
                ps = ps_big.tile([128, 512], F32, tag="big")
                for cc in range(CC):
                    nc.tensor.matmul(ps[:, :n], wqk_sb[:, cc, :],
                                     xc_t[:, cc, :n], start=(cc == 0), stop=False)
                nc.tensor.matmul(ps[:, :n], bqk_sb[:, :], ones_sb[:, :n],
                                 start=False, stop=True)
                nc.scalar.activation(qk_sb[:, p0:p0 + n], ps[:, :n], AF.Copy)
                # mirror k (partitions 64-127) down to partitions 0-63 so
                # energy matmuls get equal base partitions for both operands
                nc.sync.dma_start(out=k_sb[:, p0:p0 + n],
                                  in_=qk_sb[CQ:128, p0:p0 + n])

            # ---- column phase ----
            for x0, gd in _groups(W, GD):
                xtc_t = xtcp.tile([W, GD, C], BF16)
                nc.sync.dma_start(out=xtc_t[:, :gd, :], in_=xt_col[:, x0:x0 + gd, :])
                for s0, g in _groups(gd, GP):
                    x1 = x0 + s0
                    psA = ps_eA.tile([W, GP, W], F32, tag="eA")
                    psT = ps_eT.tile([W, GP, W], F32, tag="eT")
                    for gi in range(g):
                        x = x1 + gi
                        q_col = qk3[0:CQ, 0:R, x]
                        k_col = k3[:, :, x]
                        nc.tensor.matmul(psA[0:R, gi, :], q_col, k_col)
                        nc.tensor.matmul(psT[:, gi, 0:R], k_col, q_col)
                    pt = ptp.tile([W, GP, W], BF16, tag="pt")
                    nc.scalar.activation(pt[:, :g, 0:R], psT[:, :g, 0:R], AF.Exp)
                    nc.gpsimd.affine_select(
                        pt[:, :g, 0:R], pt[:, :g, 0:R],
                        pattern=[[0, g], [-1, R]], compare_op=ALU.not_equal,
                        fill=0.0, base=0, channel_multiplier=1)
                    scr = scrp.tile([W, GP, W], F32, tag="scr")
                    nc.scalar.activation(scr[0:R, :g, :], psA[0:R, :g, :], AF.Exp)
                    nc.gpsimd.affine_select(
                        scr[0:R, :g, :], scr[0:R, :g, :],
                        pattern=[[0, g], [-1, W]], compare_op=ALU.not_equal,
                        fill=0.0, base=0, channel_multiplier=1)
                    nc.vector.reduce_sum(scol[:, x1:x1 + g], scr[0:R, :g, :],
                                         axis=AX.X)
                    for gi in range(g):
                        x = x1 + gi
                        psG = ps_agg.tile([128, CC, W], F32, tag="agg")
                        for cc in range(CC):
                            nc.tensor.matmul(psG[:, cc, 0:R],
                                             xtc_t[:, s0 + gi, cc * 128:(cc + 1) * 128],
                                             pt[:, gi, 0:R])
                        cg = cgp.tile([128, CC, R], BF16)
                        nc.vector.tensor_copy(cg, psG[:, :, 0:R])
                        nc.gpsimd.tensor_copy(z4[:, :, :, x], cg)

            # Scol^T  (one PE transpose)
            psS = ps_eA.tile([W, R], F32, tag="eA")
            nc.tensor.transpose(psS, scol[:, :], ident[0:R, 0:R])
            nc.vector.tensor_copy(scolT, psS)

            # ---- row phase ----
            for j0, gd in _groups(R, GD):
                xtr_t = xtrp.tile([W, GD, C], BF16)
                nc.sync.dma_start(out=xtr_t[:, :gd, :], in_=xt_row[:, j0:j0 + gd, :])
                for s0, g in _groups(gd, GP):
                    j1 = j0 + s0
                    xr_t = xrp.tile([W, GP, C], BF16)
                    nc.sync.dma_start(out=xr_t[:, :g, :], in_=xr_v[:, j1:j1 + g, :])
                    psA = ps_eA.tile([W, GP, W], F32, tag="eA")
                    psT = ps_eT.tile([W, GP, W], F32, tag="eT")
                    for gi in range(g):
                        j = j1 + gi
                        q_row = qk_sb[0:CQ, j * W:(j + 1) * W]
                        k_row = k_sb[:, j * W:(j + 1) * W]
                        nc.tensor.matmul(psA[:, gi, :], q_row, k_row)
                        nc.tensor.matmul(psT[:, gi, :], k_row, q_row)
                    pt = ptp.tile([W, GP, W], BF16, tag="pt")
                    nc.scalar.activation(pt[:, :g, :], psT[:, :g, :], AF.Exp)
                    scr = scrp.tile([W, GP, W], F32, tag="scr")
                    nc.scalar.activation(scr[:, :g, :], psA[:, :g, :], AF.Exp)
                    nc.vector.reduce_sum(srow[:, j1:j1 + g], scr[:, :g, :],
                                         axis=AX.X)
                    s_t = smallp.tile([W, GP], F32)
                    nc.vector.tensor_add(s_t[:, :g], srow[:, j1:j1 + g],
                                         scolT[:, j1:j1 + g])
                    nc.vector.reciprocal(s_t[:, :g], s_t[:, :g])
                    nc.vector.tensor_scalar_mul(s_t[:, :g], s_t[:, :g], gamma)
                    outst = outp.tile([W, GP, C], F32)
                    for gi in range(g):
                        j = j1 + gi
                        psG = ps_agg.tile([128, CC, W], F32, tag="agg")
                        for cc in range(CC):
                            nc.tensor.matmul(psG[:, cc, :],
                                             xtr_t[:, s0 + gi, cc * 128:(cc + 1) * 128],
                                             pt[:, gi, :])
                        nc.vector.tensor_add(z_sb[:, :, j * W:(j + 1) * W], psG,
                                             z_sb[:, :, j * W:(j + 1) * W])
                        psF = ps_big.tile([128, 512], F32, tag="big")
                        for cc in range(CC):
                            nc.tensor.matmul(psF[0:W, :],
                                             z_sb[:, cc, j * W:(j + 1) * W],
                                             wv_sb[:, cc, :],
                                             start=(cc == 0), stop=(cc == 3))
                        nc.scalar.activation(outst[:, gi, :], psF[0:W, :],
                                             AF.Copy, scale=s_t[:, gi:gi + 1])
                        nc.vector.tensor_add(outst[:, gi, :], outst[:, gi, :],
                                             xr_t[:, gi, :])
                    nc.sync.dma_start(out=out_v[:, j1:j1 + g, :],
                                      in_=outst[:, :g, :])

    nc.compile()
    return nc


def _prep_core(x, wq, bq, wk, bk, wv, bv, gamma, n, half):
    y0 = half * 48
    xs = np.roll(x[n], -y0, axis=1)  # [C, H, W] fp32
    xc_h = np.ascontiguousarray(
        xs.reshape(CC, 128, NPIX).transpose(1, 0, 2)).astype(np.float16)
    xt_h = np.ascontiguousarray(
        xs.transpose(2, 1, 0).reshape(NPIX, C)).astype(ml_dtypes.bfloat16)
    xr_h = np.ascontiguousarray(
        (xs[:, :R, :] + gamma * bv[:, None, None]).transpose(1, 2, 0)
    ).astype(ml_dtypes.bfloat16)
    return {"xc": xc_h, "xt": xt_h, "xr": xr_h}


def kernel(x, wq, bq, wk, bk, wv, bv, gamma):
    global last_results
    x = np.asarray(x, dtype=np.float32)
    gamma_f = float(np.asarray(gamma).reshape(-1)[0])

    if "nc" not in _cache:
        _cache["nc"] = _build(gamma_f)
    nc = _cache["nc"]

    wqk_h = np.ascontiguousarray(
        np.concatenate([np.asarray(wq).T, np.asarray(wk).T], axis=1)
        .reshape(CC, 128, 128).transpose(1, 0, 2)).astype(np.float16)
    bqk_h = np.concatenate([np.asarray(bq), np.asarray(bk)])[None, :].astype(np.float16)
    ones_h = np.ones((1, 512), np.float16)
    wv_h = np.ascontiguousarray(
        np.asarray(wv).T.reshape(CC, 128, C).transpose(1, 0, 2)
    ).astype(ml_dtypes.bfloat16)

    in_maps = []
    for core in range(8):
        m = _prep_core(x, wq, bq, wk, bk, np.asarray(wv), np.asarray(bv),
                       gamma_f, core // 2, core % 2)
        m.update({"wqk": wqk_h, "bqk": bqk_h, "onesd": ones_h, "wv": wv_h})
        in_maps.append(m)

    last_results = run_bass_kernel_spmd(
        nc, in_maps, core_ids=list(range(8)),
        trace=os.environ.get("KERNEL_TRACE") == "1")

    full = np.empty((4, C, H, W), np.float32)
    for core in range(8):
        n, half = core // 2, core % 2
        y0 = half * 48
        o = last_results.results[core]["out"]  # [R, W, C]
        rows = (np.arange(R) + y0) % H
        full[n][:, rows, :] = o.transpose(2, 0, 1)
    return full



# revision 9
# speedup vs baseline: 1.3390x; 1.3390x over previous
"""CrissCrossAttention Trainium2 kernel.

Sharding: 8 cores = 4 samples x 2 row-halves. Each core gets its sample with
rows rolled so its 49 rows sit at local rows [0, 49) (column attention is
permutation-invariant over the column index, so rolling is safe and keeps the
SPMD program identical across cores).

Math (per core, local rows j in [0,49), cols x in [0,97)):
  q|k = Wqk @ x + bqk                       (fp16 matmuls, PSUM fp32)
  E_row[(j,x), i] = q(j,x).k(j,i)           (row energies, both orientations)
  E_col[(j,x), i] = q(j,x).k(i,x)           (col energies, both orientations)
  P = exp(E) unnormalized (no max-shift; |E| <~ 45 fits fp32), col diag i==j
  zeroed via affine_select. S = sum P (row incl self + col excl self).
  U = P_row @ X_row + P_col @ X_col         (bf16 aggregation of raw x)
  out = (Wv @ U) * (gamma/S) + (x + gamma*bv)   (Wv after aggregation;
  softmax scale and bias folded in at the end - exact by linearity)
"""

import os
import numpy as np
import ml_dtypes

import concourse.bacc as bacc
import concourse.bass as bass
import concourse.tile as tile
import concourse.mybir as mybir
from concourse.bass_utils import run_bass_kernel_spmd
from concourse.masks import make_identity

F32 = mybir.dt.float32
F16 = mybir.dt.float16
BF16 = mybir.dt.bfloat16
AF = mybir.ActivationFunctionType
AX = mybir.AxisListType
ALU = mybir.AluOpType

C = 512
CC = 4          # 4 chunks of 128 channels
CQ = 64
H = W = 97
NPIX = H * W    # 9409
R = 49          # rows per core (halves overlap at row 48)
PX = R * W      # 4753
GP = 4          # process group (cols / rows per softmax group)
GD = 8          # dma group for xt slices

_cache = {}
last_results = None


def _groups(total, g):
    out = []
    i = 0
    while i < total:
        out.append((i, min(g, total - i)))
        i += g
    return out


def _build(gamma: float):
    nc = bacc.Bacc("TRN2", target_bir_lowering=False, debug=False,
                   enable_asserts=False)

    xc = nc.dram_tensor("xc", [128, CC, NPIX], F16, kind="ExternalInput")
    xt = nc.dram_tensor("xt", [NPIX, C], BF16, kind="ExternalInput")
    xr = nc.dram_tensor("xr", [R, W, C], BF16, kind="ExternalInput")
    wqk = nc.dram_tensor("wqk", [128, CC, 128], F16, kind="ExternalInput")
    bqk = nc.dram_tensor("bqk", [1, 128], F16, kind="ExternalInput")
    onesd = nc.dram_tensor("onesd", [1, 512], F16, kind="ExternalInput")
    wv = nc.dram_tensor("wv", [128, CC, C], BF16, kind="ExternalInput")
    out = nc.dram_tensor("out", [R, W, C], F32, kind="ExternalOutput")

    xt_col = xt.ap().rearrange("(x y) c -> y x c", y=W)   # [y, x, c] part=y
    xt_row = xt.ap().rearrange("(x y) c -> x y c", y=W)   # [i=x, j=y, c] part=i
    xr_v = xr.ap().rearrange("j x c -> x j c")            # part = x
    out_v = out.ap().rearrange("j x c -> x j c")

    with tile.TileContext(nc) as tc:
        with (
            tc.tile_pool(name="singles", bufs=1) as singles,
            tc.tile_pool(name="xcp", bufs=3) as xcp,
            tc.tile_pool(name="xtcp", bufs=3) as xtcp,
            tc.tile_pool(name="xtrp", bufs=3) as xtrp,
            tc.tile_pool(name="xrp", bufs=3) as xrp,
            tc.tile_pool(name="outp", bufs=2) as outp,
            tc.tile_pool(name="scrp", bufs=2) as scrp,
            tc.tile_pool(name="ptp", bufs=4) as ptp,
            tc.tile_pool(name="cgp", bufs=3) as cgp,
            tc.tile_pool(name="smallp", bufs=4) as smallp,
            tc.tile_pool(name="ps_eA", bufs=2, space="PSUM") as ps_eA,
            tc.tile_pool(name="ps_eT", bufs=2, space="PSUM") as ps_eT,
            tc.tile_pool(name="ps_agg", bufs=2, space="PSUM") as ps_agg,
            tc.tile_pool(name="ps_big", bufs=2, space="PSUM") as ps_big,
        ):
            # ---- constants ----
            wqk_sb = singles.tile([128, CC, 128], F16)
            nc.sync.dma_start(out=wqk_sb, in_=wqk.ap())
            bqk_sb = singles.tile([1, 128], F16)
            nc.sync.dma_start(out=bqk_sb, in_=bqk.ap())
            ones_sb = singles.tile([1, 512], F16)
            nc.sync.dma_start(out=ones_sb, in_=onesd.ap())
            wv_sb = singles.tile([128, CC, C], BF16)
            nc.sync.dma_start(out=wv_sb, in_=wv.ap())
            ident = singles.tile([64, 64], F32)
            make_identity(nc, ident)

            qk_sb = singles.tile([128, NPIX], F16)
            k_sb = singles.tile([CQ, NPIX], F16)
            z_sb = singles.tile([128, CC, PX], BF16)
            srow = singles.tile([W, R], F32)
            scol = singles.tile([R, W], F32)
            scolT = singles.tile([W, R], F32)

            qk3 = qk_sb.rearrange("p (y x) -> p y x", x=W)
            k3 = k_sb.rearrange("p (y x) -> p y x", x=W)
            z4 = z_sb.rearrange("p c (y x) -> p c y x", x=W)

            # ---- projections: qk = [wq|wk] @ x + bqk (fp16) ----
            for p0, n in _groups(NPIX, 512):
                xc_t = xcp.tile([128, CC, 512], F16)
                nc.gpsimd.dma_start(out=xc_t[:, :, :n], in_=xc.ap()[:, :, p0:p0 + n])
                ps = ps_big.tile([128, 512], F32, tag="big")
                for cc in range(CC):
                    nc.tensor.matmul(ps[:, :n], wqk_sb[:, cc, :],
                                     xc_t[:, cc, :n], start=(cc == 0), stop=False)
                nc.tensor.matmul(ps[:, :n], bqk_sb[:, :], ones_sb[:, :n],
                                 start=False, stop=True)
                nc.scalar.activation(qk_sb[:, p0:p0 + n], ps[:, :n], AF.Copy)
                # mirror k (partitions 64-127) down to partitions 0-63 so
                # energy matmuls get equal base partitions for both operands
                nc.scalar.dma_start(out=k_sb[:, p0:p0 + n],
                                    in_=qk_sb[CQ:128, p0:p0 + n])

            # ---- column phase ----
            for x0, gd in _groups(W, GD):
                xtc_t = xtcp.tile([W, GD, C], BF16)
                nc.gpsimd.dma_start(out=xtc_t[:, :gd, :], in_=xt_col[:, x0:x0 + gd, :])
                for s0, g in _groups(gd, GP):
                    x1 = x0 + s0
                    psA = ps_eA.tile([W, GP, W], F32, tag="eA")
                    psT = ps_eT.tile([W, GP, W], F32, tag="eT")
                    for gi in range(g):
                        x = x1 + gi
                        q_col = qk3[0:CQ, 0:R, x]
                        k_col = k3[:, :, x]
                        nc.tensor.matmul(psA[0:R, gi, :], q_col, k_col)
                        nc.tensor.matmul(psT[:, gi, 0:R], k_col, q_col)
                    pt = ptp.tile([W, GP, W], BF16, tag="pt")
                    nc.scalar.activation(pt[:, :g, 0:R], psT[:, :g, 0:R], AF.Exp)
                    nc.gpsimd.affine_select(
                        pt[:, :g, 0:R], pt[:, :g, 0:R],
                        pattern=[[0, g], [-1, R]], compare_op=ALU.not_equal,
                        fill=0.0, base=0, channel_multiplier=1)
                    scr = scrp.tile([W, GP, W], F32, tag="scr")
                    nc.scalar.activation(scr[0:R, :g, :], psA[0:R, :g, :], AF.Exp)
                    nc.gpsimd.affine_select(
                        scr[0:R, :g, :], scr[0:R, :g, :],
                        pattern=[[0, g], [-1, W]], compare_op=ALU.not_equal,
                        fill=0.0, base=0, channel_multiplier=1)
                    nc.vector.reduce_sum(scol[:, x1:x1 + g], scr[0:R, :g, :],
                                         axis=AX.X)
                    for gi in range(g):
                        x = x1 + gi
                        psG = ps_agg.tile([128, CC, W], F32, tag="agg")
                        for cc in range(CC):
                            nc.tensor.matmul(psG[:, cc, 0:R],
                                             xtc_t[:, s0 + gi, cc * 128:(cc + 1) * 128],
                                             pt[:, gi, 0:R])
                        cg = cgp.tile([128, CC, R], BF16)
                        nc.vector.tensor_copy(cg, psG[:, :, 0:R])
                        nc.vector.tensor_copy(z4[:, :, :, x], cg)

            # Scol^T  (one PE transpose)
            psS = ps_eA.tile([W, R], F32, tag="eA")
            nc.tensor.transpose(psS, scol[:, :], ident[0:R, 0:R])
            nc.vector.tensor_copy(scolT, psS)

            # ---- row phase ----
            for j0, gd in _groups(R, GD):
                xtr_t = xtrp.tile([W, GD, C], BF16)
                nc.gpsimd.dma_start(out=xtr_t[:, :gd, :], in_=xt_row[:, j0:j0 + gd, :])
                for s0, g in _groups(gd, GP):
                    j1 = j0 + s0
                    xr_t = xrp.tile([W, GP, C], BF16)
                    nc.gpsimd.dma_start(out=xr_t[:, :g, :], in_=xr_v[:, j1:j1 + g, :])
                    psA = ps_eA.tile([W, GP, W], F32, tag="eA")
                    psT = ps_eT.tile([W, GP, W], F32, tag="eT")
                    for gi in range(g):
                        j = j1 + gi
                        q_row = qk_sb[0:CQ, j * W:(j + 1) * W]
                        k_row = k_sb[:, j * W:(j + 1) * W]
                        nc.tensor.matmul(psA[:, gi, :], q_row, k_row)
                        nc.tensor.matmul(psT[:, gi, :], k_row, q_row)
                    pt = ptp.tile([W, GP, W], BF16, tag="pt")
                    nc.scalar.activation(pt[:, :g, :], psT[:, :g, :], AF.Exp)
                    scr = scrp.tile([W, GP, W], F32, tag="scr")
                    nc.scalar.activation(scr[:, :g, :], psA[:, :g, :], AF.Exp)
                    nc.vector.reduce_sum(srow[:, j1:j1 + g], scr[:, :g, :],
                                         axis=AX.X)
                    s_t = smallp.tile([W, GP], F32)
                    nc.vector.tensor_add(s_t[:, :g], srow[:, j1:j1 + g],
                                         scolT[:, j1:j1 + g])
                    nc.vector.reciprocal(s_t[:, :g], s_t[:, :g])
                    nc.vector.tensor_scalar_mul(s_t[:, :g], s_t[:, :g], gamma)
                    outst = outp.tile([W, GP, C], F32)
                    for gi in range(g):
                        j = j1 + gi
                        psG = ps_agg.tile([128, CC, W], F32, tag="agg")
                        for cc in range(CC):
                            nc.tensor.matmul(psG[:, cc, :],
                                             xtr_t[:, s0 + gi, cc * 128:(cc + 1) * 128],
                                             pt[:, gi, :])
                        nc.vector.tensor_add(z_sb[:, :, j * W:(j + 1) * W], psG,
                                             z_sb[:, :, j * W:(j + 1) * W])
                        psF = ps_big.tile([128, 512], F32, tag="big")
                        for cc in range(CC):
                            nc.tensor.matmul(psF[0:W, :],
                                             z_sb[:, cc, j * W:(j + 1) * W],
                                             wv_sb[:, cc, :],
                                             start=(cc == 0), stop=(cc == 3))
                        nc.scalar.activation(outst[:, gi, :], psF[0:W, :],
                                             AF.Copy, scale=s_t[:, gi:gi + 1])
                        nc.vector.tensor_add(outst[:, gi, :], outst[:, gi, :],
                                             xr_t[:, gi, :])
                    nc.gpsimd.dma_start(out=out_v[:, j1:j1 + g, :],
                                        in_=outst[:, :g, :])

    nc.compile()
    return nc


def _prep_core(x, wq, bq, wk, bk, wv, bv, gamma, n, half):
    y0 = half * 48
    xs = np.roll(x[n], -y0, axis=1)  # [C, H, W] fp32
    xc_h = np.ascontiguousarray(
        xs.reshape(CC, 128, NPIX).transpose(1, 0, 2)).astype(np.float16)
    xt_h = np.ascontiguousarray(
        xs.transpose(2, 1, 0).reshape(NPIX, C)).astype(ml_dtypes.bfloat16)
    xr_h = np.ascontiguousarray(
        (xs[:, :R, :] + gamma * bv[:, None, None]).transpose(1, 2, 0)
    ).astype(ml_dtypes.bfloat16)
    return {"xc": xc_h, "xt": xt_h, "xr": xr_h}


def kernel(x, wq, bq, wk, bk, wv, bv, gamma):
    global last_results
    x = np.asarray(x, dtype=np.float32)
    gamma_f = float(np.asarray(gamma).reshape(-1)[0])

    if "nc" not in _cache:
        _cache["nc"] = _build(gamma_f)
    nc = _cache["nc"]

    wqk_h = np.ascontiguousarray(
        np.concatenate([np.asarray(wq).T, np.asarray(wk).T], axis=1)
        .reshape(CC, 128, 128).transpose(1, 0, 2)).astype(np.float16)
    bqk_h = np.concatenate([np.asarray(bq), np.asarray(bk)])[None, :].astype(np.float16)
    ones_h = np.ones((1, 512), np.float16)
    wv_h = np.ascontiguousarray(
        np.asarray(wv).T.reshape(CC, 128, C).transpose(1, 0, 2)
    ).astype(ml_dtypes.bfloat16)

    in_maps = []
    for core in range(8):
        m = _prep_core(x, wq, bq, wk, bk, np.asarray(wv), np.asarray(bv),
                       gamma_f, core // 2, core % 2)
        m.update({"wqk": wqk_h, "bqk": bqk_h, "onesd": ones_h, "wv": wv_h})
        in_maps.append(m)

    last_results = run_bass_kernel_spmd(
        nc, in_maps, core_ids=list(range(8)),
        trace=os.environ.get("KERNEL_TRACE") == "1")

    full = np.empty((4, C, H, W), np.float32)
    for core in range(8):
        n, half = core // 2, core % 2
        y0 = half * 48
        o = last_results.results[core]["out"]  # [R, W, C]
        rows = (np.arange(R) + y0) % H
        full[n][:, rows, :] = o.transpose(2, 0, 1)
    return full



# revision 15
# speedup vs baseline: 2.1781x; 1.6267x over previous
"""CrissCrossAttention Trainium2 kernel (v2).

Sharding: 8 cores = 4 samples x 2 row-halves. Each core's sample is rolled so
its 49 rows sit at local rows [0, 49) (column attention is permutation-
invariant over the column index, so rolling is safe and keeps the SPMD
program identical across cores).

Math (per core, local rows j in [0,49), cols x in [0,97)):
  q|k = Wqk @ x + bqk                  (fp16 matmuls, PSUM fp32)
  E_row[(j,x), i] = q(j,x).k(j,i)      E_col[(j,x), i] = q(j,x).k(i,x)
  col diag (i==j) masked to -300 in PSUM pre-exp; P = exp(E) unnormalized
  (no max-shift; |E| <~ 45 fits fp32); S accumulated by the exp's accum_out.
  U = P_row @ X_row + P_col @ X_col    (bf16, channel-major in z)
  out = (Wv @ U + bv x S) * (gamma/S) + x     (rank-1 bias matmul makes the
  final stage a pure scale-and-add; exact by linearity)

DMA strategy: all bulk transfers ride SWDGE (nc.gpsimd) which sprays
descriptors across all 16 SDMA engines; host-side layouts are chosen so
every transfer has multi-KB contiguous per-partition lines.  The k mirror
(partition shift) uses the scalar HWDGE ring to stay off the sync ring.
"""

import os
import numpy as np
import ml_dtypes

import concourse.bacc as bacc
import concourse.bass as bass
import concourse.tile as tile
import concourse.mybir as mybir
from concourse.bass_utils import run_bass_kernel_spmd
from concourse.masks import make_identity

F32 = mybir.dt.float32
F16 = mybir.dt.float16
BF16 = mybir.dt.bfloat16
AF = mybir.ActivationFunctionType
AX = mybir.AxisListType
ALU = mybir.AluOpType

C = 512
CC = 4          # 4 chunks of 128 channels
CQ = 64
H = W = 97
NPIX = H * W    # 9409
R = 49          # rows per core (halves overlap at row 48)
PX = R * W      # 4753
GP = 4          # softmax/agg group
GDC = 12        # xtc chunk (x's per dma)
GDR = 8         # xtr chunk (j's per dma)
SLAB = 512      # projection slab (pixels per dma)
T4 = 512        # phase-4 pixel tile

_cache = {}
last_results = None


def _groups(total, g):
    out = []
    i = 0
    while i < total:
        out.append((i, min(g, total - i)))
        i += g
    return out


def _build(gamma: float):
    nc = bacc.Bacc("TRN2", target_bir_lowering=False, debug=False,
                   enable_asserts=False)

    xcd = nc.dram_tensor("xcd", [128, NPIX, CC], F16, kind="ExternalInput")
    xtcd = nc.dram_tensor("xtcd", [W, W, C], BF16, kind="ExternalInput")
    xtrd = nc.dram_tensor("xtrd", [W, R, C], BF16, kind="ExternalInput")
    xresd = nc.dram_tensor("xresd", [128, PX, CC], F16, kind="ExternalInput")
    wqk = nc.dram_tensor("wqk", [128, CC, 128], F16, kind="ExternalInput")
    bqk = nc.dram_tensor("bqk", [1, 128], F16, kind="ExternalInput")
    onesd = nc.dram_tensor("onesd", [1, C], F16, kind="ExternalInput")
    wv4d = nc.dram_tensor("wv4d", [128, CC, CC, 128], BF16, kind="ExternalInput")
    bvrd = nc.dram_tensor("bvrd", [1, CC, 128], BF16, kind="ExternalInput")
    out = nc.dram_tensor("out", [128, PX, CC], F32, kind="ExternalOutput")
    dbg = None
    if os.environ.get("KERNEL_DEBUG") == "1":
        dbg = {
            "d_qk": nc.dram_tensor("d_qk", [128, NPIX], F16, kind="ExternalOutput"),
            "d_scol": nc.dram_tensor("d_scol", [R, W], F32, kind="ExternalOutput"),
            "d_srow": nc.dram_tensor("d_srow", [W, R], F32, kind="ExternalOutput"),
            "d_stot": nc.dram_tensor("d_stot", [R, W], F32, kind="ExternalOutput"),
            "d_sflat": nc.dram_tensor("d_sflat", [1, PX], BF16, kind="ExternalOutput"),
            "d_recflat": nc.dram_tensor("d_recflat", [1, PX], F32, kind="ExternalOutput"),
            "d_sbr": nc.dram_tensor("d_sbr", [128, PX], BF16, kind="ExternalOutput"),
            "d_z": nc.dram_tensor("d_z", [128, CC, PX], BF16, kind="ExternalOutput"),
        }

    with tile.TileContext(nc) as tc:
        with (
            tc.tile_pool(name="singles", bufs=1) as singles,
            tc.tile_pool(name="xcp", bufs=3) as xcp,
            tc.tile_pool(name="xtcp", bufs=2) as xtcp,
            tc.tile_pool(name="xtrp", bufs=2) as xtrp,
            tc.tile_pool(name="xresp", bufs=3) as xresp,
            tc.tile_pool(name="scrap", bufs=2) as scrap,
            tc.tile_pool(name="ptp", bufs=3) as ptp,
            tc.tile_pool(name="outp", bufs=2) as outp,
            tc.tile_pool(name="ps_e1", bufs=2, space="PSUM") as ps_e1,
            tc.tile_pool(name="ps_e2", bufs=2, space="PSUM") as ps_e2,
            tc.tile_pool(name="ps_g", bufs=2, space="PSUM") as ps_g,
        ):
            # ---- constants ----
            wqk_sb = singles.tile([128, CC, 128], F16)
            nc.sync.dma_start(out=wqk_sb, in_=wqk.ap())
            bqk_sb = singles.tile([1, 128], F16)
            nc.sync.dma_start(out=bqk_sb, in_=bqk.ap())
            ones_sb = singles.tile([1, C], F16)
            nc.sync.dma_start(out=ones_sb, in_=onesd.ap())
            wv4_sb = singles.tile([128, CC, CC, 128], BF16)
            nc.sync.dma_start(out=wv4_sb, in_=wv4d.ap())
            bvr_sb = singles.tile([1, CC, 128], BF16)
            nc.sync.dma_start(out=bvr_sb, in_=bvrd.ap())
            ident = singles.tile([W, W], F32)
            make_identity(nc, ident)
            ones1 = singles.tile([1, 128], F32)
            nc.vector.memset(ones1, 1.0)

            qk_sb = singles.tile([128, NPIX], F16)
            k_sb = singles.tile([CQ, NPIX], F16)
            z_sb = singles.tile([128, CC, PX], BF16)
            scol = singles.tile([R, W], F32)     # col S sums  [j, x]
            srow = singles.tile([W, R], F32)     # row S sums  [x, j]
            stot = singles.tile([R, W], F32)
            sflat = singles.tile([1, PX], BF16)
            recs = singles.tile([R, W], F32)
            recflat = singles.tile([1, PX], F32)
            sbr = singles.tile([128, PX], BF16)  # gamma/S bcast to 128 parts

            qk3 = qk_sb.rearrange("p (y x) -> p y x", x=W)
            k3 = k_sb.rearrange("p (y x) -> p y x", x=W)

            # ---- projections: qk = [wq|wk] @ x + bqk (fp16) ----
            for p0, n in _groups(NPIX, SLAB):
                xc_t = xcp.tile([128, SLAB, CC], F16)
                nc.gpsimd.dma_start(out=xc_t[:, :n, :], in_=xcd.ap()[:, p0:p0 + n, :])
                for q0, m in _groups(n, 512):
                    ps = ps_g.tile([128, 512], F32, tag="g")
                    for cc in range(CC):
                        nc.tensor.matmul(ps[:, :m], wqk_sb[:, cc, :],
                                         xc_t[:, q0:q0 + m, cc],
                                         start=(cc == 0), stop=False)
                    nc.tensor.matmul(ps[:, :m], bqk_sb[:, :], ones_sb[:, :m],
                                     start=False, stop=True)
                    nc.scalar.activation(qk_sb[:, p0 + q0:p0 + q0 + m],
                                         ps[:, :m], AF.Copy)
                # mirror k (partitions 64-127) down to partitions 0-63 via the
                # scalar HWDGE ring (sync ring stays free for semaphores)
                nc.scalar.dma_start(out=k_sb[:, p0:p0 + n],
                                    in_=qk_sb[CQ:128, p0:p0 + n])

            # ---- column phase ----
            for x0, gd in _groups(W, GDC):
                xtc_t = xtcp.tile([W, GDC, C], BF16)
                nc.gpsimd.dma_start(out=xtc_t[:, :gd, :], in_=xtcd.ap()[:, x0:x0 + gd, :])
                for s0, g in _groups(gd, GP):
                    x1 = x0 + s0
                    psA = ps_e1.tile([R, GP, W], F32, tag="e1")
                    psT = ps_e2.tile([W, GP, R], F32, tag="e2")
                    for gi in range(g):
                        x = x1 + gi
                        q_col = qk3[0:CQ, 0:R, x]
                        k_col = k3[:, :, x]
                        nc.tensor.matmul(psA[:, gi, :], q_col, k_col)
                        nc.tensor.matmul(psT[:, gi, :], k_col, q_col)
                    scr = scrap.tile([R, GP, W], F32, tag="scr")
                    nc.scalar.activation(scr[:, :g, :], psA[:, :g, :], AF.Exp)
                    nc.gpsimd.affine_select(
                        scr[:, :g, :], scr[:, :g, :],
                        pattern=[[0, g], [-1, W]], compare_op=ALU.not_equal,
                        fill=0.0, base=0, channel_multiplier=1)
                    nc.vector.reduce_sum(scol[:, x1:x1 + g], scr[:, :g, :],
                                         axis=AX.X)
                    pt = ptp.tile([W, GP, R], BF16, tag="pt")
                    nc.scalar.activation(pt[:, :g, :], psT[:, :g, :], AF.Exp)
                    nc.gpsimd.affine_select(
                        pt[:, :g, :], pt[:, :g, :],
                        pattern=[[0, g], [-1, R]], compare_op=ALU.not_equal,
                        fill=0.0, base=0, channel_multiplier=1)
                    psG = ps_g.tile([128, CC, GP, R], F32, tag="g")
                    for gi in range(g):
                        x = x1 + gi
                        for cc in range(CC):
                            nc.tensor.matmul(
                                psG[:, cc, gi, :],
                                xtc_t[:, s0 + gi, cc * 128:(cc + 1) * 128],
                                pt[:, gi, :])
                        nc.vector.tensor_copy(
                            z_sb.rearrange("p c (y x) -> p c y x", x=W)[:, :, :, x],
                            psG[:, :, gi, :])

            # ---- row phase ----
            for j0, gd in _groups(R, GDR):
                xtr_t = xtrp.tile([W, GDR, C], BF16)
                nc.gpsimd.dma_start(out=xtr_t[:, :gd, :], in_=xtrd.ap()[:, j0:j0 + gd, :])
                for s0, g in _groups(gd, GP):
                    j1 = j0 + s0
                    psA = ps_e1.tile([W, GP, W], F32, tag="e1")
                    psT = ps_e2.tile([W, GP, W], F32, tag="e2")
                    for gi in range(g):
                        j = j1 + gi
                        q_row = qk_sb[0:CQ, j * W:(j + 1) * W]
                        k_row = k_sb[:, j * W:(j + 1) * W]
                        nc.tensor.matmul(psA[:, gi, :], q_row, k_row)
                        nc.tensor.matmul(psT[:, gi, :], k_row, q_row)
                    scr = scrap.tile([W, GP, W], F32, tag="scr")
                    for gi in range(g):
                        nc.scalar.activation(scr[:, gi, :], psA[:, gi, :],
                                             AF.Exp,
                                             accum_out=srow[:, j1 + gi:j1 + gi + 1])
                    pt = ptp.tile([W, GP, W], BF16, tag="pt")
                    nc.scalar.activation(pt[:, :g, :], psT[:, :g, :], AF.Exp)
                    for gi in range(g):
                        j = j1 + gi
                        psG = ps_g.tile([128, CC, W], F32, tag="g")
                        for cc in range(CC):
                            nc.tensor.matmul(
                                psG[:, cc, :],
                                xtr_t[:, s0 + gi, cc * 128:(cc + 1) * 128],
                                pt[:, gi, :])
                        nc.vector.tensor_add(z_sb[:, :, j * W:(j + 1) * W], psG,
                                             z_sb[:, :, j * W:(j + 1) * W])

            # ---- S merge: stot = scol + srow^T; recs = gamma / stot ----
            psS = ps_e1.tile([R, W], F32, tag="e1")
            nc.tensor.transpose(psS, srow[:, :], ident[:, :])
            nc.vector.tensor_add(stot, scol, psS)
            nc.vector.reciprocal(recs, stot)
            nc.vector.tensor_scalar_mul(recs, recs, gamma)
            # flatten [49, 97] grids to [1, PX] pixel vectors (SWDGE casts)
            nc.gpsimd.dma_start(out=sflat.rearrange("p (j x) -> p j x", x=W),
                                in_=stot[:, :])
            nc.gpsimd.dma_start(out=recflat.rearrange("p (j x) -> p j x", x=W),
                                in_=recs[:, :])
            # broadcast gamma/S to all 128 partitions
            for t0, tn in _groups(PX, T4):
                psB = ps_g.tile([128, T4], F32, tag="g")
                nc.tensor.matmul(psB[:, :tn], ones1[:, :], recflat[:, t0:t0 + tn])
                nc.vector.tensor_copy(sbr[:, t0:t0 + tn], psB[:, :tn])

            if dbg is not None:
                nc.sync.dma_start(out=dbg["d_qk"].ap(), in_=qk_sb)
                nc.sync.dma_start(out=dbg["d_scol"].ap(), in_=scol)
                nc.sync.dma_start(out=dbg["d_srow"].ap(), in_=srow)
                nc.sync.dma_start(out=dbg["d_stot"].ap(), in_=stot)
                nc.sync.dma_start(out=dbg["d_sflat"].ap(), in_=sflat)
                nc.sync.dma_start(out=dbg["d_recflat"].ap(), in_=recflat)
                nc.sync.dma_start(out=dbg["d_sbr"].ap(), in_=sbr)
                nc.sync.dma_start(out=dbg["d_z"].ap(), in_=z_sb)

            # ---- phase 4: out = (Wv@U + bv x S) * (gamma/S) + x ----
            for t0, tn in _groups(PX, T4):
                xr_t = xresp.tile([128, T4, CC], F16)
                nc.gpsimd.dma_start(out=xr_t[:, :tn, :], in_=xresd.ap()[:, t0:t0 + tn, :])
                outst = outp.tile([128, T4, CC], F32)
                for cco in range(CC):
                    psO = ps_g.tile([128, T4], F32, tag="g")
                    for cci in range(CC):
                        nc.tensor.matmul(psO[:, :tn], wv4_sb[:, cci, cco, :],
                                         z_sb[:, cci, t0:t0 + tn],
                                         start=(cci == 0), stop=False)
                    nc.tensor.matmul(psO[:, :tn], bvr_sb[:, cco, :],
                                     sflat[:, t0:t0 + tn],
                                     start=False, stop=True)
                    nc.vector.tensor_mul(outst[:, :tn, cco], psO[:, :tn],
                                         sbr[:, t0:t0 + tn])
                    nc.vector.tensor_add(outst[:, :tn, cco], outst[:, :tn, cco],
                                         xr_t[:, :tn, cco])
                nc.gpsimd.dma_start(out=out.ap()[:, t0:t0 + tn, :],
                                    in_=outst[:, :tn, :])

    nc.compile()
    return nc


def _prep_core(x, n, half):
    y0 = half * 48
    xs = np.roll(x[n], -y0, axis=1)  # [C, H, W] fp32
    xcd_h = np.ascontiguousarray(
        xs.reshape(CC, 128, NPIX).transpose(1, 2, 0)).astype(np.float16)
    xtcd_h = np.ascontiguousarray(
        xs.transpose(1, 2, 0)).astype(ml_dtypes.bfloat16)
    xtrd_h = np.ascontiguousarray(
        xs[:, :R, :].transpose(2, 1, 0)).astype(ml_dtypes.bfloat16)
    xresd_h = np.ascontiguousarray(
        xs[:, :R, :].reshape(CC, 128, PX).transpose(1, 2, 0)).astype(np.float16)
    return {"xcd": xcd_h, "xtcd": xtcd_h, "xtrd": xtrd_h, "xresd": xresd_h}


def kernel(x, wq, bq, wk, bk, wv, bv, gamma):
    global last_results
    x = np.asarray(x, dtype=np.float32)
    gamma_f = float(np.asarray(gamma).reshape(-1)[0])

    if "nc" not in _cache:
        _cache["nc"] = _build(gamma_f)
    nc = _cache["nc"]

    wqk_h = np.ascontiguousarray(
        np.concatenate([np.asarray(wq).T, np.asarray(wk).T], axis=1)
        .reshape(CC, 128, 128).transpose(1, 0, 2)).astype(np.float16)
    bqk_h = np.concatenate([np.asarray(bq), np.asarray(bk)])[None, :].astype(np.float16)
    ones_h = np.ones((1, C), np.float16)
    wv4_h = np.ascontiguousarray(
        np.asarray(wv).T.reshape(CC, 128, CC, 128).transpose(1, 0, 2, 3)
    ).astype(ml_dtypes.bfloat16)
    bvr_h = np.asarray(bv).reshape(CC, 128)[None].astype(ml_dtypes.bfloat16)

    shared = {"wqk": wqk_h, "bqk": bqk_h, "onesd": ones_h, "wv4d": wv4_h,
              "bvrd": bvr_h}
    in_maps = []
    for core in range(8):
        m = _prep_core(x, core // 2, core % 2)
        m.update(shared)
        in_maps.append(m)

    last_results = run_bass_kernel_spmd(
        nc, in_maps, core_ids=list(range(8)),
        trace=os.environ.get("KERNEL_TRACE") == "1")

    full = np.empty((4, C, H, W), np.float32)
    for core in range(8):
        n, half = core // 2, core % 2
        y0 = half * 48
        o = last_results.results[core]["out"]  # [128, PX, CC]
        rows = (np.arange(R) + y0) % H
        full[n][:, rows, :] = o.transpose(2, 0, 1).reshape(C, R, W)
    return full


# revision 18
# speedup vs baseline: 4.1560x; 1.9080x over previous
"""CrissCrossAttention Trainium2 kernel (v2).

Sharding: 8 cores = 4 samples x 2 row-halves. Each core's sample is rolled so
its 49 rows sit at local rows [0, 49) (column attention is permutation-
invariant over the column index, so rolling is safe and keeps the SPMD
program identical across cores).

Math (per core, local rows j in [0,49), cols x in [0,97)):
  q|k = Wqk @ x + bqk                  (fp16 matmuls, PSUM fp32)
  E_row[(j,x), i] = q(j,x).k(j,i)      E_col[(j,x), i] = q(j,x).k(i,x)
  col diag (i==j) masked to -300 in PSUM pre-exp; P = exp(E) unnormalized
  (no max-shift; |E| <~ 45 fits fp32); S accumulated by the exp's accum_out.
  U = P_row @ X_row + P_col @ X_col    (bf16, channel-major in z)
  out = (Wv @ U + bv x S) * (gamma/S) + x     (rank-1 bias matmul makes the
  final stage a pure scale-and-add; exact by linearity)

DMA strategy: all bulk transfers ride SWDGE (nc.gpsimd) which sprays
descriptors across all 16 SDMA engines; host-side layouts are chosen so
every transfer has multi-KB contiguous per-partition lines.  The k mirror
(partition shift) uses the scalar HWDGE ring to stay off the sync ring.
"""

import os
import numpy as np
import ml_dtypes

import concourse.bacc as bacc
import concourse.bass as bass
import concourse.tile as tile
import concourse.mybir as mybir
from concourse.bass_utils import run_bass_kernel_spmd
from concourse.masks import make_identity

F32 = mybir.dt.float32
F16 = mybir.dt.float16
BF16 = mybir.dt.bfloat16
AF = mybir.ActivationFunctionType
AX = mybir.AxisListType
ALU = mybir.AluOpType

C = 512
CC = 4          # 4 chunks of 128 channels
CQ = 64
H = W = 97
NPIX = H * W    # 9409
R = 49          # rows per core (halves overlap at row 48)
PX = R * W      # 4753
GP = 4          # softmax/agg group
GDC = 12        # xtc chunk (x's per dma)
GDR = 8         # xtr chunk (j's per dma)
SLAB = 512      # projection slab (pixels per dma)
T4 = 512        # phase-4 pixel tile

_cache = {}
last_results = None


def _groups(total, g):
    out = []
    i = 0
    while i < total:
        out.append((i, min(g, total - i)))
        i += g
    return out


def _build(gamma: float):
    nc = bacc.Bacc("TRN2", target_bir_lowering=False, debug=False,
                   enable_asserts=False)

    xcd = nc.dram_tensor("xcd", [128, NPIX, CC], F16, kind="ExternalInput")
    xtcd = nc.dram_tensor("xtcd", [128, W, C], BF16, kind="ExternalInput")
    xtrd = nc.dram_tensor("xtrd", [128, R, C], BF16, kind="ExternalInput")
    xresd = nc.dram_tensor("xresd", [128, PX, CC], F16, kind="ExternalInput")
    wqk = nc.dram_tensor("wqk", [128, CC, 128], F16, kind="ExternalInput")
    bqk = nc.dram_tensor("bqk", [1, 128], F16, kind="ExternalInput")
    onesd = nc.dram_tensor("onesd", [1, C], F16, kind="ExternalInput")
    wv4d = nc.dram_tensor("wv4d", [128, CC, CC, 128], BF16, kind="ExternalInput")
    bvrd = nc.dram_tensor("bvrd", [1, CC, 128], BF16, kind="ExternalInput")
    out = nc.dram_tensor("out", [128, PX, CC], F32, kind="ExternalOutput")
    dbg = None
    if os.environ.get("KERNEL_DEBUG") == "1":
        dbg = {
            "d_qk": nc.dram_tensor("d_qk", [128, NPIX], F16, kind="ExternalOutput"),
            "d_scol": nc.dram_tensor("d_scol", [R, W], F32, kind="ExternalOutput"),
            "d_srow": nc.dram_tensor("d_srow", [W, R], F32, kind="ExternalOutput"),
            "d_stot": nc.dram_tensor("d_stot", [R, W], F32, kind="ExternalOutput"),
            "d_sflat": nc.dram_tensor("d_sflat", [1, PX], BF16, kind="ExternalOutput"),
            "d_recflat": nc.dram_tensor("d_recflat", [1, PX], F32, kind="ExternalOutput"),
            "d_sbr": nc.dram_tensor("d_sbr", [128, PX], BF16, kind="ExternalOutput"),
            "d_z": nc.dram_tensor("d_z", [128, CC, PX], BF16, kind="ExternalOutput"),
        }

    with tile.TileContext(nc) as tc:
        with (
            tc.tile_pool(name="singles", bufs=1) as singles,
            tc.tile_pool(name="xcp", bufs=3) as xcp,
            tc.tile_pool(name="xtcp", bufs=2) as xtcp,
            tc.tile_pool(name="xtrp", bufs=2) as xtrp,
            tc.tile_pool(name="xresp", bufs=3) as xresp,
            tc.tile_pool(name="scrap", bufs=2) as scrap,
            tc.tile_pool(name="ptp", bufs=3) as ptp,
            tc.tile_pool(name="outp", bufs=2) as outp,
            tc.tile_pool(name="ps_e1", bufs=2, space="PSUM") as ps_e1,
            tc.tile_pool(name="ps_e2", bufs=2, space="PSUM") as ps_e2,
            tc.tile_pool(name="ps_g", bufs=2, space="PSUM") as ps_g,
        ):
            # ---- constants ----
            wqk_sb = singles.tile([128, CC, 128], F16)
            nc.sync.dma_start(out=wqk_sb, in_=wqk.ap())
            bqk_sb = singles.tile([1, 128], F16)
            nc.sync.dma_start(out=bqk_sb, in_=bqk.ap())
            ones_sb = singles.tile([1, C], F16)
            nc.sync.dma_start(out=ones_sb, in_=onesd.ap())
            wv4_sb = singles.tile([128, CC, CC, 128], BF16)
            nc.sync.dma_start(out=wv4_sb, in_=wv4d.ap())
            bvr_sb = singles.tile([1, CC, 128], BF16)
            nc.sync.dma_start(out=bvr_sb, in_=bvrd.ap())
            ident = singles.tile([W, W], F32)
            make_identity(nc, ident)
            ones1 = singles.tile([1, 128], F32)
            nc.vector.memset(ones1, 1.0)

            qk_sb = singles.tile([128, NPIX], F16)
            k_sb = singles.tile([CQ, NPIX], F16)
            z_sb = singles.tile([128, CC, PX], BF16)
            scol = singles.tile([R, W], F32)     # col S sums  [j, x]
            srow = singles.tile([W, R], F32)     # row S sums  [x, j]
            stot = singles.tile([R, W], F32)
            sflat = singles.tile([1, PX], BF16)
            recs = singles.tile([R, W], F32)
            recflat = singles.tile([1, PX], F32)
            sbr = singles.tile([128, PX], BF16)  # gamma/S bcast to 128 parts

            qk3 = qk_sb.rearrange("p (y x) -> p y x", x=W)
            k3 = k_sb.rearrange("p (y x) -> p y x", x=W)

            # ---- projections: qk = [wq|wk] @ x + bqk (fp16) ----
            for p0, n in _groups(NPIX, SLAB):
                xc_t = xcp.tile([128, SLAB, CC], F16)
                nc.gpsimd.dma_start(out=xc_t[:, :n, :], in_=xcd.ap()[:, p0:p0 + n, :])
                for q0, m in _groups(n, 512):
                    ps = ps_g.tile([128, 512], F32, tag="g")
                    for cc in range(CC):
                        nc.tensor.matmul(ps[:, :m], wqk_sb[:, cc, :],
                                         xc_t[:, q0:q0 + m, cc],
                                         start=(cc == 0), stop=False)
                    nc.tensor.matmul(ps[:, :m], bqk_sb[:, :], ones_sb[:, :m],
                                     start=False, stop=True)
                    nc.scalar.activation(qk_sb[:, p0 + q0:p0 + q0 + m],
                                         ps[:, :m], AF.Copy)
                # mirror k (partitions 64-127) down to partitions 0-63 via the
                # scalar HWDGE ring (sync ring stays free for semaphores)
                nc.scalar.dma_start(out=k_sb[:, p0:p0 + n],
                                    in_=qk_sb[CQ:128, p0:p0 + n])

            # ---- column phase ----
            for x0, gd in _groups(W, GDC):
                xtc_t = xtcp.tile([128, GDC, C], BF16)
                nc.gpsimd.dma_start(out=xtc_t[:, :gd, :], in_=xtcd.ap()[:, x0:x0 + gd, :])
                for s0, g in _groups(gd, GP):
                    x1 = x0 + s0
                    psA = ps_e1.tile([R, GP, W], F32, tag="e1")
                    psT = ps_e2.tile([W, GP, R], F32, tag="e2")
                    for gi in range(g):
                        x = x1 + gi
                        q_col = qk3[0:CQ, 0:R, x]
                        k_col = k3[:, :, x]
                        nc.tensor.matmul(psA[:, gi, :], q_col, k_col)
                        nc.tensor.matmul(psT[:, gi, :], k_col, q_col)
                    scr = scrap.tile([R, GP, W], F32, tag="scr")
                    nc.scalar.activation(scr[:, :g, :], psA[:, :g, :], AF.Exp)
                    nc.gpsimd.affine_select(
                        scr[:, :g, :], scr[:, :g, :],
                        pattern=[[0, g], [-1, W]], compare_op=ALU.not_equal,
                        fill=0.0, base=0, channel_multiplier=1)
                    nc.vector.reduce_sum(scol[:, x1:x1 + g], scr[:, :g, :],
                                         axis=AX.X)
                    pt = ptp.tile([128, GP, R], BF16, tag="pt")
                    nc.vector.memset(pt[96:128, :, :], 0.0)
                    nc.scalar.activation(pt[0:W, :g, :], psT[:, :g, :], AF.Exp)
                    nc.gpsimd.affine_select(
                        pt[0:W, :g, :], pt[0:W, :g, :],
                        pattern=[[0, g], [-1, R]], compare_op=ALU.not_equal,
                        fill=0.0, base=0, channel_multiplier=1)
                    psG = ps_g.tile([128, CC, GP, R], F32, tag="g")
                    for gi in range(g):
                        x = x1 + gi
                        for cc in range(CC):
                            nc.tensor.matmul(
                                psG[:, cc, gi, :],
                                xtc_t[:, s0 + gi, cc * 128:(cc + 1) * 128],
                                pt[:, gi, :])
                        nc.vector.tensor_copy(
                            z_sb.rearrange("p c (y x) -> p c y x", x=W)[:, :, :, x],
                            psG[:, :, gi, :])

            # ---- row phase ----
            for j0, gd in _groups(R, GDR):
                xtr_t = xtrp.tile([128, GDR, C], BF16)
                nc.gpsimd.dma_start(out=xtr_t[:, :gd, :], in_=xtrd.ap()[:, j0:j0 + gd, :])
                for s0, g in _groups(gd, GP):
                    j1 = j0 + s0
                    psA = ps_e1.tile([W, GP, W], F32, tag="e1")
                    psT = ps_e2.tile([W, GP, W], F32, tag="e2")
                    for gi in range(g):
                        j = j1 + gi
                        q_row = qk_sb[0:CQ, j * W:(j + 1) * W]
                        k_row = k_sb[:, j * W:(j + 1) * W]
                        nc.tensor.matmul(psA[:, gi, :], q_row, k_row)
                        nc.tensor.matmul(psT[:, gi, :], k_row, q_row)
                    scr = scrap.tile([W, GP, W], F32, tag="scr")
                    for gi in range(g):
                        nc.scalar.activation(scr[:, gi, :], psA[:, gi, :],
                                             AF.Exp,
                                             accum_out=srow[:, j1 + gi:j1 + gi + 1])
                    pt = ptp.tile([128, GP, W], BF16, tag="pt")
                    nc.vector.memset(pt[96:128, :, :], 0.0)
                    nc.scalar.activation(pt[0:W, :g, :], psT[:, :g, :], AF.Exp)
                    for gi in range(g):
                        j = j1 + gi
                        psG = ps_g.tile([128, CC, W], F32, tag="g")
                        for cc in range(CC):
                            nc.tensor.matmul(
                                psG[:, cc, :],
                                xtr_t[:, s0 + gi, cc * 128:(cc + 1) * 128],
                                pt[:, gi, :])
                        nc.vector.tensor_add(z_sb[:, :, j * W:(j + 1) * W], psG,
                                             z_sb[:, :, j * W:(j + 1) * W])

            # ---- S merge: stot = scol + srow^T; recs = gamma / stot ----
            psS = ps_e1.tile([R, W], F32, tag="e1")
            nc.tensor.transpose(psS, srow[:, :], ident[:, :])
            nc.vector.tensor_add(stot, scol, psS)
            nc.vector.reciprocal(recs, stot)
            nc.vector.tensor_scalar_mul(recs, recs, gamma)
            # flatten [49, 97] grids to [1, PX] pixel vectors (SWDGE casts)
            nc.gpsimd.dma_start(out=sflat.rearrange("p (j x) -> p j x", x=W),
                                in_=stot[:, :])
            nc.gpsimd.dma_start(out=recflat.rearrange("p (j x) -> p j x", x=W),
                                in_=recs[:, :])
            # broadcast gamma/S to all 128 partitions
            for t0, tn in _groups(PX, T4):
                psB = ps_g.tile([128, T4], F32, tag="g")
                nc.tensor.matmul(psB[:, :tn], ones1[:, :], recflat[:, t0:t0 + tn])
                nc.vector.tensor_copy(sbr[:, t0:t0 + tn], psB[:, :tn])

            if dbg is not None:
                nc.sync.dma_start(out=dbg["d_qk"].ap(), in_=qk_sb)
                nc.sync.dma_start(out=dbg["d_scol"].ap(), in_=scol)
                nc.sync.dma_start(out=dbg["d_srow"].ap(), in_=srow)
                nc.sync.dma_start(out=dbg["d_stot"].ap(), in_=stot)
                nc.sync.dma_start(out=dbg["d_sflat"].ap(), in_=sflat)
                nc.sync.dma_start(out=dbg["d_recflat"].ap(), in_=recflat)
                nc.sync.dma_start(out=dbg["d_sbr"].ap(), in_=sbr)
                nc.sync.dma_start(out=dbg["d_z"].ap(), in_=z_sb)

            # ---- phase 4: out = (Wv@U + bv x S) * (gamma/S) + x ----
            for t0, tn in _groups(PX, T4):
                xr_t = xresp.tile([128, T4, CC], F16)
                nc.gpsimd.dma_start(out=xr_t[:, :tn, :], in_=xresd.ap()[:, t0:t0 + tn, :])
                outst = outp.tile([128, T4, CC], F32)
                for cco in range(CC):
                    psO = ps_g.tile([128, T4], F32, tag="g")
                    for cci in range(CC):
                        nc.tensor.matmul(psO[:, :tn], wv4_sb[:, cci, cco, :],
                                         z_sb[:, cci, t0:t0 + tn],
                                         start=(cci == 0), stop=False)
                    nc.tensor.matmul(psO[:, :tn], bvr_sb[:, cco, :],
                                     sflat[:, t0:t0 + tn],
                                     start=False, stop=True)
                    nc.vector.tensor_mul(outst[:, :tn, cco], psO[:, :tn],
                                         sbr[:, t0:t0 + tn])
                    nc.vector.tensor_add(outst[:, :tn, cco], outst[:, :tn, cco],
                                         xr_t[:, :tn, cco])
                nc.gpsimd.dma_start(out=out.ap()[:, t0:t0 + tn, :],
                                    in_=outst[:, :tn, :])

    nc.compile()
    return nc


def _prep_core(x, n, half):
    y0 = half * 48
    xs = np.roll(x[n], -y0, axis=1)  # [C, H, W] fp32
    xcd_h = np.ascontiguousarray(
        xs.reshape(CC, 128, NPIX).transpose(1, 2, 0)).astype(np.float16)
    # padded to 128 partitions (rows 97-127 zero) so agg stationaries hit FWL
    xtcd_h = np.zeros((128, W, C), ml_dtypes.bfloat16)
    xtcd_h[:W] = xs.transpose(1, 2, 0).astype(ml_dtypes.bfloat16)
    xtrd_h = np.zeros((128, R, C), ml_dtypes.bfloat16)
    xtrd_h[:W] = xs[:, :R, :].transpose(2, 1, 0).astype(ml_dtypes.bfloat16)
    xresd_h = np.ascontiguousarray(
        xs[:, :R, :].reshape(CC, 128, PX).transpose(1, 2, 0)).astype(np.float16)
    return {"xcd": xcd_h, "xtcd": xtcd_h, "xtrd": xtrd_h, "xresd": xresd_h}


def kernel(x, wq, bq, wk, bk, wv, bv, gamma):
    global last_results
    x = np.asarray(x, dtype=np.float32)
    gamma_f = float(np.asarray(gamma).reshape(-1)[0])

    if "nc" not in _cache:
        _cache["nc"] = _build(gamma_f)
    nc = _cache["nc"]

    wqk_h = np.ascontiguousarray(
        np.concatenate([np.asarray(wq).T, np.asarray(wk).T], axis=1)
        .reshape(CC, 128, 128).transpose(1, 0, 2)).astype(np.float16)
    bqk_h = np.concatenate([np.asarray(bq), np.asarray(bk)])[None, :].astype(np.float16)
    ones_h = np.ones((1, C), np.float16)
    wv4_h = np.ascontiguousarray(
        np.asarray(wv).T.reshape(CC, 128, CC, 128).transpose(1, 0, 2, 3)
    ).astype(ml_dtypes.bfloat16)
    bvr_h = np.asarray(bv).reshape(CC, 128)[None].astype(ml_dtypes.bfloat16)

    shared = {"wqk": wqk_h, "bqk": bqk_h, "onesd": ones_h, "wv4d": wv4_h,
              "bvrd": bvr_h}
    in_maps = []
    for core in range(8):
        m = _prep_core(x, core // 2, core % 2)
        m.update(shared)
        in_maps.append(m)

    last_results = run_bass_kernel_spmd(
        nc, in_maps, core_ids=list(range(8)),
        trace=os.environ.get("KERNEL_TRACE") == "1")

    full = np.empty((4, C, H, W), np.float32)
    for core in range(8):
        n, half = core // 2, core % 2
        y0 = half * 48
        o = last_results.results[core]["out"]  # [128, PX, CC]
        rows = (np.arange(R) + y0) % H
        full[n][:, rows, :] = o.transpose(2, 0, 1).reshape(C, R, W)
    return full


# revision 20
# speedup vs baseline: 4.5997x; 1.1068x over previous
"""CrissCrossAttention Trainium2 kernel (v2).

Sharding: 8 cores = 4 samples x 2 row-halves. Each core's sample is rolled so
its 49 rows sit at local rows [0, 49) (column attention is permutation-
invariant over the column index, so rolling is safe and keeps the SPMD
program identical across cores).

Math (per core, local rows j in [0,49), cols x in [0,97)):
  q|k = Wqk @ x + bqk                  (fp16 matmuls, PSUM fp32)
  E_row[(j,x), i] = q(j,x).k(j,i)      E_col[(j,x), i] = q(j,x).k(i,x)
  col diag (i==j) masked to -300 in PSUM pre-exp; P = exp(E) unnormalized
  (no max-shift; |E| <~ 45 fits fp32); S accumulated by the exp's accum_out.
  U = P_row @ X_row + P_col @ X_col    (bf16, channel-major in z)
  out = (Wv @ U + bv x S) * (gamma/S) + x     (rank-1 bias matmul makes the
  final stage a pure scale-and-add; exact by linearity)

DMA strategy: all bulk transfers ride SWDGE (nc.gpsimd) which sprays
descriptors across all 16 SDMA engines; host-side layouts are chosen so
every transfer has multi-KB contiguous per-partition lines.  The k mirror
(partition shift) uses the scalar HWDGE ring to stay off the sync ring.
"""

import os
import numpy as np
import ml_dtypes

import concourse.bacc as bacc
import concourse.bass as bass
import concourse.tile as tile
import concourse.mybir as mybir
from concourse.bass_utils import run_bass_kernel_spmd
from concourse.masks import make_identity

F32 = mybir.dt.float32
F16 = mybir.dt.float16
BF16 = mybir.dt.bfloat16
AF = mybir.ActivationFunctionType
AX = mybir.AxisListType
ALU = mybir.AluOpType

C = 512
CC = 4          # 4 chunks of 128 channels
CQ = 64
H = W = 97
NPIX = H * W    # 9409
R = 49          # rows per core (halves overlap at row 48)
PX = R * W      # 4753
GP = 4          # softmax/agg group
GDC = 12        # xtc chunk (x's per dma)
GDR = 8         # xtr chunk (j's per dma)
SLAB = 512      # projection slab (pixels per dma)
T4 = 512        # phase-4 pixel tile

_cache = {}
last_results = None


def _groups(total, g):
    out = []
    i = 0
    while i < total:
        out.append((i, min(g, total - i)))
        i += g
    return out


def _build(gamma: float):
    nc = bacc.Bacc("TRN2", target_bir_lowering=False, debug=False,
                   enable_asserts=False)

    xcd = nc.dram_tensor("xcd", [128, NPIX, CC], F16, kind="ExternalInput")
    xtcd = nc.dram_tensor("xtcd", [128, W, C], BF16, kind="ExternalInput")
    xtrd = nc.dram_tensor("xtrd", [128, R, C], BF16, kind="ExternalInput")
    xresd = nc.dram_tensor("xresd", [128, CC, PX], F16, kind="ExternalInput")
    wqk = nc.dram_tensor("wqk", [128, CC, 128], F16, kind="ExternalInput")
    bqk = nc.dram_tensor("bqk", [1, 128], F16, kind="ExternalInput")
    onesd = nc.dram_tensor("onesd", [1, C], F16, kind="ExternalInput")
    wv4d = nc.dram_tensor("wv4d", [128, CC, CC, 128], BF16, kind="ExternalInput")
    gbvd = nc.dram_tensor("gbvd", [128, CC], F32, kind="ExternalInput")
    out = nc.dram_tensor("out", [128, CC, PX], F32, kind="ExternalOutput")
    dbg = None
    if os.environ.get("KERNEL_DEBUG") == "1":
        dbg = {
            "d_qk": nc.dram_tensor("d_qk", [128, NPIX], F16, kind="ExternalOutput"),
            "d_scol": nc.dram_tensor("d_scol", [R, W], F32, kind="ExternalOutput"),
            "d_srow": nc.dram_tensor("d_srow", [W, R], F32, kind="ExternalOutput"),
            "d_stot": nc.dram_tensor("d_stot", [R, W], F32, kind="ExternalOutput"),
            "d_recflat": nc.dram_tensor("d_recflat", [1, PX], F32, kind="ExternalOutput"),
            "d_sbr": nc.dram_tensor("d_sbr", [128, PX], BF16, kind="ExternalOutput"),
            "d_z": nc.dram_tensor("d_z", [128, CC, PX], BF16, kind="ExternalOutput"),
        }

    with tile.TileContext(nc) as tc:
        with (
            tc.tile_pool(name="singles", bufs=1) as singles,
            tc.tile_pool(name="xcp", bufs=3) as xcp,
            tc.tile_pool(name="xtcp", bufs=2) as xtcp,
            tc.tile_pool(name="xtrp", bufs=2) as xtrp,
            tc.tile_pool(name="xresp", bufs=3) as xresp,
            tc.tile_pool(name="scrap", bufs=2) as scrap,
            tc.tile_pool(name="ptp", bufs=3) as ptp,
            tc.tile_pool(name="outp", bufs=2) as outp,
            tc.tile_pool(name="ps_e1", bufs=2, space="PSUM") as ps_e1,
            tc.tile_pool(name="ps_e2", bufs=2, space="PSUM") as ps_e2,
            tc.tile_pool(name="ps_g", bufs=2, space="PSUM") as ps_g,
        ):
            # ---- constants ----
            wqk_sb = singles.tile([128, CC, 128], F16)
            nc.sync.dma_start(out=wqk_sb, in_=wqk.ap())
            bqk_sb = singles.tile([1, 128], F16)
            nc.sync.dma_start(out=bqk_sb, in_=bqk.ap())
            ones_sb = singles.tile([1, C], F16)
            nc.sync.dma_start(out=ones_sb, in_=onesd.ap())
            wv4_sb = singles.tile([128, CC, CC, 128], BF16)
            nc.sync.dma_start(out=wv4_sb, in_=wv4d.ap())
            gbv_sb = singles.tile([128, CC], F32)
            nc.sync.dma_start(out=gbv_sb, in_=gbvd.ap())
            ident = singles.tile([W, W], F32)
            make_identity(nc, ident)
            ones1 = singles.tile([1, 128], F32)
            nc.vector.memset(ones1, 1.0)

            qk_sb = singles.tile([128, NPIX], F16)
            k_sb = singles.tile([CQ, NPIX], F16)
            z_sb = singles.tile([128, CC, PX], BF16)
            scol = singles.tile([R, W], F32)     # col S sums  [j, x]
            srow = singles.tile([W, R], F32)     # row S sums  [x, j]
            stot = singles.tile([R, W], F32)
            recs = singles.tile([R, W], F32)
            recflat = singles.tile([1, PX], F32)
            sbr = singles.tile([128, PX], BF16)  # gamma/S bcast to 128 parts

            qk3 = qk_sb.rearrange("p (y x) -> p y x", x=W)
            k3 = k_sb.rearrange("p (y x) -> p y x", x=W)

            # ---- projections: qk = [wq|wk] @ x + bqk (fp16) ----
            for p0, n in _groups(NPIX, SLAB):
                xc_t = xcp.tile([128, SLAB, CC], F16)
                nc.gpsimd.dma_start(out=xc_t[:, :n, :], in_=xcd.ap()[:, p0:p0 + n, :])
                for q0, m in _groups(n, 512):
                    ps = ps_g.tile([128, 512], F32, tag="g")
                    for cc in range(CC):
                        nc.tensor.matmul(ps[:, :m], wqk_sb[:, cc, :],
                                         xc_t[:, q0:q0 + m, cc],
                                         start=(cc == 0), stop=False)
                    nc.tensor.matmul(ps[:, :m], bqk_sb[:, :], ones_sb[:, :m],
                                     start=False, stop=True)
                    nc.scalar.activation(qk_sb[:, p0 + q0:p0 + q0 + m],
                                         ps[:, :m], AF.Copy)
                # mirror k (partitions 64-127) down to partitions 0-63 via the
                # scalar HWDGE ring (sync ring stays free for semaphores)
                nc.scalar.dma_start(out=k_sb[:, p0:p0 + n],
                                    in_=qk_sb[CQ:128, p0:p0 + n])

            # ---- column phase ----
            for x0, gd in _groups(W, GDC):
                xtc_t = xtcp.tile([128, GDC, C], BF16)
                nc.gpsimd.dma_start(out=xtc_t[:, :gd, :], in_=xtcd.ap()[:, x0:x0 + gd, :])
                for s0, g in _groups(gd, GP):
                    x1 = x0 + s0
                    psA = ps_e1.tile([R, GP, W], F32, tag="e1")
                    psT = ps_e2.tile([W, GP, R], F32, tag="e2")
                    for gi in range(g):
                        x = x1 + gi
                        q_col = qk3[0:CQ, 0:R, x]
                        k_col = k3[:, :, x]
                        nc.tensor.matmul(psA[:, gi, :], q_col, k_col)
                        nc.tensor.matmul(psT[:, gi, :], k_col, q_col)
                    scr = scrap.tile([R, GP, W], F32, tag="scr")
                    nc.scalar.activation(scr[:, :g, :], psA[:, :g, :], AF.Exp)
                    nc.gpsimd.affine_select(
                        scr[:, :g, :], scr[:, :g, :],
                        pattern=[[0, g], [-1, W]], compare_op=ALU.not_equal,
                        fill=0.0, base=0, channel_multiplier=1)
                    nc.vector.reduce_sum(scol[:, x1:x1 + g], scr[:, :g, :],
                                         axis=AX.X)
                    pt = ptp.tile([128, GP, R], BF16, tag="pt")
                    nc.vector.memset(pt[96:128, :, :], 0.0)
                    nc.scalar.activation(pt[0:W, :g, :], psT[:, :g, :], AF.Exp)
                    nc.gpsimd.affine_select(
                        pt[0:W, :g, :], pt[0:W, :g, :],
                        pattern=[[0, g], [-1, R]], compare_op=ALU.not_equal,
                        fill=0.0, base=0, channel_multiplier=1)
                    psG = ps_g.tile([128, CC, GP, R], F32, tag="g")
                    for gi in range(g):
                        x = x1 + gi
                        for cc in range(CC):
                            nc.tensor.matmul(
                                psG[:, cc, gi, :],
                                xtc_t[:, s0 + gi, cc * 128:(cc + 1) * 128],
                                pt[:, gi, :])
                        zv = z_sb.rearrange("p c (y x) -> p c y x", x=W)[:, :, :, x]
                        if x % 2 == 0:
                            nc.scalar.activation(zv, psG[:, :, gi, :], AF.Copy)
                        else:
                            nc.vector.tensor_copy(zv, psG[:, :, gi, :])

            # ---- row phase ----
            for j0, gd in _groups(R, GDR):
                xtr_t = xtrp.tile([128, GDR, C], BF16)
                nc.gpsimd.dma_start(out=xtr_t[:, :gd, :], in_=xtrd.ap()[:, j0:j0 + gd, :])
                for s0, g in _groups(gd, GP):
                    j1 = j0 + s0
                    psA = ps_e1.tile([W, GP, W], F32, tag="e1")
                    psT = ps_e2.tile([W, GP, W], F32, tag="e2")
                    for gi in range(g):
                        j = j1 + gi
                        q_row = qk_sb[0:CQ, j * W:(j + 1) * W]
                        k_row = k_sb[:, j * W:(j + 1) * W]
                        nc.tensor.matmul(psA[:, gi, :], q_row, k_row)
                        nc.tensor.matmul(psT[:, gi, :], k_row, q_row)
                    scr = scrap.tile([W, GP, W], F32, tag="scr")
                    for gi in range(g):
                        nc.scalar.activation(scr[:, gi, :], psA[:, gi, :],
                                             AF.Exp,
                                             accum_out=srow[:, j1 + gi:j1 + gi + 1])
                    pt = ptp.tile([128, GP, W], BF16, tag="pt")
                    nc.vector.memset(pt[96:128, :, :], 0.0)
                    nc.scalar.activation(pt[0:W, :g, :], psT[:, :g, :], AF.Exp)
                    for gi in range(g):
                        j = j1 + gi
                        psG = ps_g.tile([128, CC, W], F32, tag="g")
                        for cc in range(CC):
                            nc.tensor.matmul(
                                psG[:, cc, :],
                                xtr_t[:, s0 + gi, cc * 128:(cc + 1) * 128],
                                pt[:, gi, :])
                        nc.vector.tensor_add(z_sb[:, :, j * W:(j + 1) * W], psG,
                                             z_sb[:, :, j * W:(j + 1) * W])

            # ---- S merge: stot = scol + srow^T; recs = gamma / stot ----
            psS = ps_e1.tile([R, W], F32, tag="e1")
            nc.tensor.transpose(psS, srow[:, :], ident[:, :])
            nc.vector.tensor_add(stot, scol, psS)
            nc.vector.reciprocal(recs, stot)
            nc.vector.tensor_scalar_mul(recs, recs, gamma)
            # flatten [49, 97] grid to a [1, PX] pixel vector
            nc.gpsimd.dma_start(out=recflat.rearrange("p (j x) -> p j x", x=W),
                                in_=recs[:, :])
            # broadcast gamma/S to all 128 partitions
            for t0, tn in _groups(PX, T4):
                psB = ps_g.tile([128, T4], F32, tag="g")
                nc.tensor.matmul(psB[:, :tn], ones1[:, :], recflat[:, t0:t0 + tn])
                nc.vector.tensor_copy(sbr[:, t0:t0 + tn], psB[:, :tn])

            if dbg is not None:
                nc.sync.dma_start(out=dbg["d_qk"].ap(), in_=qk_sb)
                nc.sync.dma_start(out=dbg["d_scol"].ap(), in_=scol)
                nc.sync.dma_start(out=dbg["d_srow"].ap(), in_=srow)
                nc.sync.dma_start(out=dbg["d_stot"].ap(), in_=stot)
                nc.sync.dma_start(out=dbg["d_recflat"].ap(), in_=recflat)
                nc.sync.dma_start(out=dbg["d_sbr"].ap(), in_=sbr)
                nc.sync.dma_start(out=dbg["d_z"].ap(), in_=z_sb)

            # ---- phase 4: out = (Wv@U + bv x S) * (gamma/S) + x ----
            for t0, tn in _groups(PX, T4):
                xr_t = xresp.tile([128, CC, T4], F16)
                nc.gpsimd.dma_start(out=xr_t[:, :, :tn], in_=xresd.ap()[:, :, t0:t0 + tn])
                outst = outp.tile([128, CC, T4], F32)
                for cco in range(CC):
                    psO = ps_g.tile([128, T4], F32, tag="g")
                    for cci in range(CC):
                        nc.tensor.matmul(psO[:, :tn], wv4_sb[:, cci, cco, :],
                                         z_sb[:, cci, t0:t0 + tn],
                                         start=(cci == 0), stop=(cci == CC - 1))
                    nc.vector.tensor_mul(outst[:, cco, :tn], psO[:, :tn],
                                         sbr[:, t0:t0 + tn])
                    # out = (psO*gamma/S + gamma*bv) + x
                    nc.vector.scalar_tensor_tensor(
                        outst[:, cco, :tn], outst[:, cco, :tn],
                        gbv_sb[:, cco:cco + 1], xr_t[:, cco, :tn],
                        op0=ALU.add, op1=ALU.add)
                nc.gpsimd.dma_start(out=out.ap()[:, :, t0:t0 + tn],
                                    in_=outst[:, :, :tn])

    nc.compile()
    return nc


def _prep_core(x, n, half):
    y0 = half * 48
    xs = np.roll(x[n], -y0, axis=1)  # [C, H, W] fp32
    xcd_h = np.ascontiguousarray(
        xs.reshape(CC, 128, NPIX).transpose(1, 2, 0)).astype(np.float16)
    # padded to 128 partitions (rows 97-127 zero) so agg stationaries hit FWL
    xtcd_h = np.zeros((128, W, C), ml_dtypes.bfloat16)
    xtcd_h[:W] = xs.transpose(1, 2, 0).astype(ml_dtypes.bfloat16)
    xtrd_h = np.zeros((128, R, C), ml_dtypes.bfloat16)
    xtrd_h[:W] = xs[:, :R, :].transpose(2, 1, 0).astype(ml_dtypes.bfloat16)
    xresd_h = np.ascontiguousarray(
        xs[:, :R, :].reshape(CC, 128, PX).transpose(1, 0, 2)).astype(np.float16)
    return {"xcd": xcd_h, "xtcd": xtcd_h, "xtrd": xtrd_h, "xresd": xresd_h}


def kernel(x, wq, bq, wk, bk, wv, bv, gamma):
    global last_results
    x = np.asarray(x, dtype=np.float32)
    gamma_f = float(np.asarray(gamma).reshape(-1)[0])

    if "nc" not in _cache:
        _cache["nc"] = _build(gamma_f)
    nc = _cache["nc"]

    wqk_h = np.ascontiguousarray(
        np.concatenate([np.asarray(wq).T, np.asarray(wk).T], axis=1)
        .reshape(CC, 128, 128).transpose(1, 0, 2)).astype(np.float16)
    bqk_h = np.concatenate([np.asarray(bq), np.asarray(bk)])[None, :].astype(np.float16)
    ones_h = np.ones((1, C), np.float16)
    wv4_h = np.ascontiguousarray(
        np.asarray(wv).T.reshape(CC, 128, CC, 128).transpose(1, 0, 2, 3)
    ).astype(ml_dtypes.bfloat16)
    gbv_h = np.ascontiguousarray(
        (gamma_f * np.asarray(bv)).reshape(CC, 128).T).astype(np.float32)

    shared = {"wqk": wqk_h, "bqk": bqk_h, "onesd": ones_h, "wv4d": wv4_h,
              "gbvd": gbv_h}
    in_maps = []
    for core in range(8):
        m = _prep_core(x, core // 2, core % 2)
        m.update(shared)
        in_maps.append(m)

    last_results = run_bass_kernel_spmd(
        nc, in_maps, core_ids=list(range(8)),
        trace=os.environ.get("KERNEL_TRACE") == "1")

    full = np.empty((4, C, H, W), np.float32)
    for core in range(8):
        n, half = core // 2, core % 2
        y0 = half * 48
        o = last_results.results[core]["out"]  # [128, CC, PX]
        rows = (np.arange(R) + y0) % H
        full[n][:, rows, :] = o.transpose(1, 0, 2).reshape(C, R, W)
    return full


# revision 21
# speedup vs baseline: 4.6592x; 1.0129x over previous
"""CrissCrossAttention Trainium2 kernel (v2).

Sharding: 8 cores = 4 samples x 2 row-halves. Each core's sample is rolled so
its 49 rows sit at local rows [0, 49) (column attention is permutation-
invariant over the column index, so rolling is safe and keeps the SPMD
program identical across cores).

Math (per core, local rows j in [0,49), cols x in [0,97)):
  q|k = Wqk @ x + bqk                  (fp16 matmuls, PSUM fp32)
  E_row[(j,x), i] = q(j,x).k(j,i)      E_col[(j,x), i] = q(j,x).k(i,x)
  col diag (i==j) masked to -300 in PSUM pre-exp; P = exp(E) unnormalized
  (no max-shift; |E| <~ 45 fits fp32); S accumulated by the exp's accum_out.
  U = P_row @ X_row + P_col @ X_col    (bf16, channel-major in z)
  out = (Wv @ U + bv x S) * (gamma/S) + x     (rank-1 bias matmul makes the
  final stage a pure scale-and-add; exact by linearity)

DMA strategy: all bulk transfers ride SWDGE (nc.gpsimd) which sprays
descriptors across all 16 SDMA engines; host-side layouts are chosen so
every transfer has multi-KB contiguous per-partition lines.  The k mirror
(partition shift) uses the scalar HWDGE ring to stay off the sync ring.
"""

import os
import numpy as np
import ml_dtypes

import concourse.bacc as bacc
import concourse.bass as bass
import concourse.tile as tile
import concourse.mybir as mybir
from concourse.bass_utils import run_bass_kernel_spmd
from concourse.masks import make_identity

F32 = mybir.dt.float32
F16 = mybir.dt.float16
BF16 = mybir.dt.bfloat16
AF = mybir.ActivationFunctionType
AX = mybir.AxisListType
ALU = mybir.AluOpType

C = 512
CC = 4          # 4 chunks of 128 channels
CQ = 64
H = W = 97
NPIX = H * W    # 9409
R = 49          # rows per core (halves overlap at row 48)
PX = R * W      # 4753
GP = 4          # softmax/agg group
GDC = 12        # xtc chunk (x's per dma)
GDR = 8         # xtr chunk (j's per dma)
SLAB = 512      # projection slab (pixels per dma)
T4 = 512        # phase-4 pixel tile

_cache = {}
last_results = None


def _groups(total, g):
    out = []
    i = 0
    while i < total:
        out.append((i, min(g, total - i)))
        i += g
    return out


def _build(gamma: float):
    nc = bacc.Bacc("TRN2", target_bir_lowering=False, debug=False,
                   enable_asserts=False)

    xcd = nc.dram_tensor("xcd", [128, NPIX, CC], F16, kind="ExternalInput")
    xtcd = nc.dram_tensor("xtcd", [128, W, C], BF16, kind="ExternalInput")
    xtrd = nc.dram_tensor("xtrd", [128, R, C], BF16, kind="ExternalInput")
    xresd = nc.dram_tensor("xresd", [128, CC, PX], F16, kind="ExternalInput")
    wqk = nc.dram_tensor("wqk", [128, CC, 128], F16, kind="ExternalInput")
    bias2 = nc.dram_tensor("bias2", [128, 128], F16, kind="ExternalInput")
    ones2 = nc.dram_tensor("ones2", [128, 512], F16, kind="ExternalInput")
    wv4d = nc.dram_tensor("wv4d", [128, CC, CC, 128], BF16, kind="ExternalInput")
    gbvd = nc.dram_tensor("gbvd", [128, CC], F32, kind="ExternalInput")
    out = nc.dram_tensor("out", [128, CC, PX], F32, kind="ExternalOutput")
    dbg = None
    if os.environ.get("KERNEL_DEBUG") == "1":
        dbg = {
            "d_qk": nc.dram_tensor("d_qk", [128, NPIX], F16, kind="ExternalOutput"),
            "d_scol": nc.dram_tensor("d_scol", [R, W], F32, kind="ExternalOutput"),
            "d_srow": nc.dram_tensor("d_srow", [W, R], F32, kind="ExternalOutput"),
            "d_stot": nc.dram_tensor("d_stot", [R, W], F32, kind="ExternalOutput"),
            "d_recflat": nc.dram_tensor("d_recflat", [1, PX], F32, kind="ExternalOutput"),
            "d_sbr": nc.dram_tensor("d_sbr", [128, PX], BF16, kind="ExternalOutput"),
            "d_z": nc.dram_tensor("d_z", [128, CC, PX], BF16, kind="ExternalOutput"),
        }

    with tile.TileContext(nc) as tc:
        with (
            tc.tile_pool(name="singles", bufs=1) as singles,
            tc.tile_pool(name="xcp", bufs=3) as xcp,
            tc.tile_pool(name="kstp", bufs=3) as kstp,
            tc.tile_pool(name="xtcp", bufs=2) as xtcp,
            tc.tile_pool(name="xtrp", bufs=2) as xtrp,
            tc.tile_pool(name="xresp", bufs=3) as xresp,
            tc.tile_pool(name="scrap", bufs=2) as scrap,
            tc.tile_pool(name="ptp", bufs=3) as ptp,
            tc.tile_pool(name="outp", bufs=2) as outp,
            tc.tile_pool(name="ps_e1", bufs=2, space="PSUM") as ps_e1,
            tc.tile_pool(name="ps_e2", bufs=2, space="PSUM") as ps_e2,
            tc.tile_pool(name="ps_g", bufs=2, space="PSUM") as ps_g,
        ):
            # ---- constants ----
            wqk_sb = singles.tile([128, CC, 128], F16)
            nc.sync.dma_start(out=wqk_sb, in_=wqk.ap())
            bias2_sb = singles.tile([128, 128], F16)
            nc.sync.dma_start(out=bias2_sb, in_=bias2.ap())
            ones2_sb = singles.tile([128, 512], F16)
            nc.sync.dma_start(out=ones2_sb, in_=ones2.ap())
            wv4_sb = singles.tile([128, CC, CC, 128], BF16)
            nc.sync.dma_start(out=wv4_sb, in_=wv4d.ap())
            gbv_sb = singles.tile([128, CC], F32)
            nc.sync.dma_start(out=gbv_sb, in_=gbvd.ap())
            ident = singles.tile([W, W], F32)
            make_identity(nc, ident)
            ones1 = singles.tile([1, 128], F32)
            nc.vector.memset(ones1, 1.0)

            q_sb = singles.tile([128, NPIX], F16)
            k_sb = singles.tile([128, NPIX], F16)
            nc.vector.memset(q_sb[CQ:128, :], 0.0)
            nc.vector.memset(k_sb[CQ:128, :], 0.0)
            z_sb = singles.tile([128, CC, PX], BF16)
            scol = singles.tile([R, W], F32)     # col S sums  [j, x]
            srow = singles.tile([W, R], F32)     # row S sums  [x, j]
            stot = singles.tile([R, W], F32)
            recs = singles.tile([R, W], F32)
            recflat = singles.tile([1, PX], F32)
            sbr = singles.tile([128, PX], BF16)  # gamma/S bcast to 128 parts

            q3 = q_sb.rearrange("p (y x) -> p y x", x=W)
            k3 = k_sb.rearrange("p (y x) -> p y x", x=W)

            # ---- projections: qk = [wq|wk] @ x + bqk (fp16) ----
            for p0, n in _groups(NPIX, SLAB):
                xc_t = xcp.tile([128, SLAB, CC], F16)
                nc.gpsimd.dma_start(out=xc_t[:, :n, :], in_=xcd.ap()[:, p0:p0 + n, :])
                for q0, m in _groups(n, 512):
                    ps = ps_g.tile([128, 512], F32, tag="g")
                    for cc in range(CC):
                        nc.tensor.matmul(ps[:, :m], wqk_sb[:, cc, :],
                                         xc_t[:, q0:q0 + m, cc],
                                         start=(cc == 0), stop=False)
                    nc.tensor.matmul(ps[:, :m], bias2_sb, ones2_sb[:, :m],
                                     start=False, stop=True)
                    nc.scalar.activation(q_sb[0:CQ, p0 + q0:p0 + q0 + m],
                                         ps[0:CQ, :m], AF.Copy)
                    kst = kstp.tile([128, 512], F16, tag="kst")
                    nc.scalar.activation(kst[CQ:128, :m], ps[CQ:128, :m], AF.Copy)
                    # mirror k (partitions 64-127) down to partitions 0-63 via
                    # the scalar HWDGE ring
                    nc.scalar.dma_start(out=k_sb[0:CQ, p0 + q0:p0 + q0 + m],
                                        in_=kst[CQ:128, :m])

            # ---- column phase ----
            for x0, gd in _groups(W, GDC):
                xtc_t = xtcp.tile([128, GDC, C], BF16)
                nc.gpsimd.dma_start(out=xtc_t[:, :gd, :], in_=xtcd.ap()[:, x0:x0 + gd, :])
                for s0, g in _groups(gd, GP):
                    x1 = x0 + s0
                    psA = ps_e1.tile([R, GP, W], F32, tag="e1")
                    psT = ps_e2.tile([W, GP, R], F32, tag="e2")
                    for gi in range(g):
                        x = x1 + gi
                        q_col = q3[:, 0:R, x]
                        k_col = k3[:, :, x]
                        nc.tensor.matmul(psA[:, gi, :], q_col, k_col)
                        nc.tensor.matmul(psT[:, gi, :], k_col, q_col)
                    scr = scrap.tile([R, GP, W], F32, tag="scr")
                    nc.scalar.activation(scr[:, :g, :], psA[:, :g, :], AF.Exp)
                    nc.gpsimd.affine_select(
                        scr[:, :g, :], scr[:, :g, :],
                        pattern=[[0, g], [-1, W]], compare_op=ALU.not_equal,
                        fill=0.0, base=0, channel_multiplier=1)
                    nc.vector.reduce_sum(scol[:, x1:x1 + g], scr[:, :g, :],
                                         axis=AX.X)
                    pt = ptp.tile([128, GP, R], BF16, tag="pt")
                    nc.vector.memset(pt[96:128, :, :], 0.0)
                    nc.scalar.activation(pt[0:W, :g, :], psT[:, :g, :], AF.Exp)
                    nc.gpsimd.affine_select(
                        pt[0:W, :g, :], pt[0:W, :g, :],
                        pattern=[[0, g], [-1, R]], compare_op=ALU.not_equal,
                        fill=0.0, base=0, channel_multiplier=1)
                    psG = ps_g.tile([128, CC, GP, R], F32, tag="g")
                    for gi in range(g):
                        x = x1 + gi
                        for cc in range(CC):
                            nc.tensor.matmul(
                                psG[:, cc, gi, :],
                                xtc_t[:, s0 + gi, cc * 128:(cc + 1) * 128],
                                pt[:, gi, :])
                        zv = z_sb.rearrange("p c (y x) -> p c y x", x=W)[:, :, :, x]
                        if x % 2 == 0:
                            nc.scalar.activation(zv, psG[:, :, gi, :], AF.Copy)
                        else:
                            nc.vector.tensor_copy(zv, psG[:, :, gi, :])

            # ---- row phase ----
            for j0, gd in _groups(R, GDR):
                xtr_t = xtrp.tile([128, GDR, C], BF16)
                nc.gpsimd.dma_start(out=xtr_t[:, :gd, :], in_=xtrd.ap()[:, j0:j0 + gd, :])
                for s0, g in _groups(gd, GP):
                    j1 = j0 + s0
                    psA = ps_e1.tile([W, GP, W], F32, tag="e1")
                    psT = ps_e2.tile([W, GP, W], F32, tag="e2")
                    for gi in range(g):
                        j = j1 + gi
                        q_row = q_sb[:, j * W:(j + 1) * W]
                        k_row = k_sb[:, j * W:(j + 1) * W]
                        nc.tensor.matmul(psA[:, gi, :], q_row, k_row)
                        nc.tensor.matmul(psT[:, gi, :], k_row, q_row)
                    scr = scrap.tile([W, GP, W], F32, tag="scr")
                    for gi in range(g):
                        nc.scalar.activation(scr[:, gi, :], psA[:, gi, :],
                                             AF.Exp,
                                             accum_out=srow[:, j1 + gi:j1 + gi + 1])
                    pt = ptp.tile([128, GP, W], BF16, tag="pt")
                    nc.vector.memset(pt[96:128, :, :], 0.0)
                    nc.scalar.activation(pt[0:W, :g, :], psT[:, :g, :], AF.Exp)
                    for gi in range(g):
                        j = j1 + gi
                        psG = ps_g.tile([128, CC, W], F32, tag="g")
                        for cc in range(CC):
                            nc.tensor.matmul(
                                psG[:, cc, :],
                                xtr_t[:, s0 + gi, cc * 128:(cc + 1) * 128],
                                pt[:, gi, :])
                        nc.vector.tensor_add(z_sb[:, :, j * W:(j + 1) * W], psG,
                                             z_sb[:, :, j * W:(j + 1) * W])

            # ---- S merge: stot = scol + srow^T; recs = gamma / stot ----
            psS = ps_e1.tile([R, W], F32, tag="e1")
            nc.tensor.transpose(psS, srow[:, :], ident[:, :])
            nc.vector.tensor_add(stot, scol, psS)
            nc.vector.reciprocal(recs, stot)
            nc.vector.tensor_scalar_mul(recs, recs, gamma)
            # flatten [49, 97] grid to a [1, PX] pixel vector
            nc.gpsimd.dma_start(out=recflat.rearrange("p (j x) -> p j x", x=W),
                                in_=recs[:, :])
            # broadcast gamma/S to all 128 partitions
            for t0, tn in _groups(PX, T4):
                psB = ps_g.tile([128, T4], F32, tag="g")
                nc.tensor.matmul(psB[:, :tn], ones1[:, :], recflat[:, t0:t0 + tn])
                nc.vector.tensor_copy(sbr[:, t0:t0 + tn], psB[:, :tn])

            if dbg is not None:
                nc.sync.dma_start(out=dbg["d_qk"].ap(), in_=q_sb)
                nc.sync.dma_start(out=dbg["d_scol"].ap(), in_=scol)
                nc.sync.dma_start(out=dbg["d_srow"].ap(), in_=srow)
                nc.sync.dma_start(out=dbg["d_stot"].ap(), in_=stot)
                nc.sync.dma_start(out=dbg["d_recflat"].ap(), in_=recflat)
                nc.sync.dma_start(out=dbg["d_sbr"].ap(), in_=sbr)
                nc.sync.dma_start(out=dbg["d_z"].ap(), in_=z_sb)

            # ---- phase 4: out = (Wv@U + bv x S) * (gamma/S) + x ----
            for t0, tn in _groups(PX, T4):
                xr_t = xresp.tile([128, CC, T4], F16)
                nc.gpsimd.dma_start(out=xr_t[:, :, :tn], in_=xresd.ap()[:, :, t0:t0 + tn])
                outst = outp.tile([128, CC, T4], F32)
                for cco in range(CC):
                    psO = ps_g.tile([128, T4], F32, tag="g")
                    for cci in range(CC):
                        nc.tensor.matmul(psO[:, :tn], wv4_sb[:, cci, cco, :],
                                         z_sb[:, cci, t0:t0 + tn],
                                         start=(cci == 0), stop=(cci == CC - 1))
                    nc.vector.tensor_mul(outst[:, cco, :tn], psO[:, :tn],
                                         sbr[:, t0:t0 + tn])
                    # out = (psO*gamma/S + gamma*bv) + x
                    nc.vector.scalar_tensor_tensor(
                        outst[:, cco, :tn], outst[:, cco, :tn],
                        gbv_sb[:, cco:cco + 1], xr_t[:, cco, :tn],
                        op0=ALU.add, op1=ALU.add)
                nc.gpsimd.dma_start(out=out.ap()[:, :, t0:t0 + tn],
                                    in_=outst[:, :, :tn])

    nc.compile()
    return nc


def _prep_core(x, n, half):
    y0 = half * 48
    xs = np.roll(x[n], -y0, axis=1)  # [C, H, W] fp32
    xcd_h = np.ascontiguousarray(
        xs.reshape(CC, 128, NPIX).transpose(1, 2, 0)).astype(np.float16)
    # padded to 128 partitions (rows 97-127 zero) so agg stationaries hit FWL
    xtcd_h = np.zeros((128, W, C), ml_dtypes.bfloat16)
    xtcd_h[:W] = xs.transpose(1, 2, 0).astype(ml_dtypes.bfloat16)
    xtrd_h = np.zeros((128, R, C), ml_dtypes.bfloat16)
    xtrd_h[:W] = xs[:, :R, :].transpose(2, 1, 0).astype(ml_dtypes.bfloat16)
    xresd_h = np.ascontiguousarray(
        xs[:, :R, :].reshape(CC, 128, PX).transpose(1, 0, 2)).astype(np.float16)
    return {"xcd": xcd_h, "xtcd": xtcd_h, "xtrd": xtrd_h, "xresd": xresd_h}


def kernel(x, wq, bq, wk, bk, wv, bv, gamma):
    global last_results
    x = np.asarray(x, dtype=np.float32)
    gamma_f = float(np.asarray(gamma).reshape(-1)[0])

    if "nc" not in _cache:
        _cache["nc"] = _build(gamma_f)
    nc = _cache["nc"]

    wqk_h = np.ascontiguousarray(
        np.concatenate([np.asarray(wq).T, np.asarray(wk).T], axis=1)
        .reshape(CC, 128, 128).transpose(1, 0, 2)).astype(np.float16)
    bias2_h = np.zeros((128, 128), np.float16)
    bias2_h[0] = np.concatenate([np.asarray(bq), np.asarray(bk)]).astype(np.float16)
    ones2_h = np.ones((128, 512), np.float16)
    wv4_h = np.ascontiguousarray(
        np.asarray(wv).T.reshape(CC, 128, CC, 128).transpose(1, 0, 2, 3)
    ).astype(ml_dtypes.bfloat16)
    gbv_h = np.ascontiguousarray(
        (gamma_f * np.asarray(bv)).reshape(CC, 128).T).astype(np.float32)

    shared = {"wqk": wqk_h, "bias2": bias2_h, "ones2": ones2_h,
              "wv4d": wv4_h, "gbvd": gbv_h}
    in_maps = []
    for core in range(8):
        m = _prep_core(x, core // 2, core % 2)
        m.update(shared)
        in_maps.append(m)

    last_results = run_bass_kernel_spmd(
        nc, in_maps, core_ids=list(range(8)),
        trace=os.environ.get("KERNEL_TRACE") == "1")

    full = np.empty((4, C, H, W), np.float32)
    for core in range(8):
        n, half = core // 2, core % 2
        y0 = half * 48
        o = last_results.results[core]["out"]  # [128, CC, PX]
        rows = (np.arange(R) + y0) % H
        full[n][:, rows, :] = o.transpose(1, 0, 2).reshape(C, R, W)
    return full
